# revision 124
# baseline (speedup 1.0000x reference)
"""Causal multi-head attention (B=4, T=2048, C=1024, H=16) on 8 Trainium2 cores.

Sharding: core c handles batch b = c//2 and heads h0..h0+7 with h0 = (c%2)*8.
Each core computes QKV projection for its head slice, causal attention for its
8 heads, and a partial output projection. Host sums the two partials per batch
and adds the bias terms.

fp8e4 DoubleRow pipeline (180us HW; bf16 baseline was 237us at 94% PE):
  - fp8e4 matmuls with MatmulPerfMode.DoubleRow process 2 contraction
    k-tiles per instruction at 0.5 cycles/column -- 4x bf16 FLOP rate
    when the contraction fills 256, 2x when it is only 64 (scores).
    QKV production, scores, and AV for strips 1-3 all run this way; the
    PE drops from 226us busy to ~124us and the Activation engine's exp
    stream (~150us busy, element count fixed by the causal mask and
    psum-bank-limited instruction sizes) becomes the binding engine.
    Layouts: q8/k8 hold each head's 64 channels as 2x32 "k-tiles" on one
    partition quadrant (32-row matmuls at tile_position 0/32/64/96); v8
    packs [t-chunk, head*68] so AV pairs adjacent k-chunks as k-tiles,
    with the softmax ones-column riding at slot 64 (M=65 out rows).
  - Accuracy: rows with few attended keys cannot average away fp8 noise,
    so strip 0 (q in [0,512), n_eff as low as 1) keeps the full bf16
    path: bf16 q/k c-tiles, bf16 v chunks 0-3, bf16 u. Strips 1-3
    (n_eff >= ~190) run raw fp8: max-abs rel err 9.3e-3 vs the 2e-2
    gate (bf16 baseline was 4.1e-3). exp writes u as fp8 with bias -2
    so e^s stays below fp8e4's 240 max; normalization cancels the bias.
  - Strip order (1, 2, 0, 3): the fp8 stream (wqk8, x8, wv8) is DMA'd
    first and strip-1 scores start ~10us in, keeping ACT saturated from
    then on; the bf16 bundle for strip 0 streams in the fp8 strips'
    shadow and its production runs as strip-1/2 filler matmuls. Strip 3
    stays last: its long exp tail hides all deferred projection work.
  - Per-pair softmax normalize: psum -> bf16 oun copies (frees the AV
    banks), then a DEFERRED chain -- e64-matmul sum extract (no DMA
    hop), bf16 reciprocal + row broadcast + multiply (DVE 2x modes) --
    emitted after the NEXT pair's first scores so the in-order PE never
    stalls behind DVE queueing at pair boundaries.
  - One psum pool set for the whole program (pool open/close is a full
    drain barrier): 2x scores [P,2,512] (4 banks) + 2x AV [65,512]
    (2 banks, rotating with the deferred sum-extract tiles) + 2 filler
    banks. The projection tail stage-majors across the same banks with
    held-back fillers bridging the tail pair's normalize latency.
  - y is stored bf16 (halves output DMA); host sums partials in f32 and
    folds the v-bias via b_v @ w_proj (softmax rows sum to 1).
"""

import os
import sys
import numpy as np

sys.path.insert(0, "/opt/trn_rl_repo")

import concourse.bass as bass  # noqa: E402
import concourse.bacc as bacc  # noqa: E402
import concourse.mybir as mybir  # noqa: E402
from concourse.bass_utils import run_bass_kernel_spmd  # noqa: E402
from concourse.tile import TileContext  # noqa: E402

B, T, C, H = 4, 2048, 1024, 16
HD = C // H          # 64 head dim
HPC = 8              # heads per core
P = 128
NT = T // P          # 16 t-chunks of 128
SW = 512             # strip width (q and t strips)
NS = T // SW         # 4 strips
KC = C // P          # 8 contraction chunks for QKV
CL = HPC * HD        # 512 local channels per section
EH = HD + 1          # 65: head slot width in v (value cols + ones col)
F32 = mybir.dt.float32
BF16 = mybir.dt.bfloat16
F8 = mybir.dt.float8e4
DR = mybir.MatmulPerfMode.DoubleRow
EV = 68          # v8 head slot: 64 v cols + ones col at 64 + 3 pad (4B align)
EXPF = mybir.ActivationFunctionType.Exp
MUL = mybir.AluOpType.mult
DIV = mybir.AluOpType.divide
EBIAS = -4.0     # exp(s*scale + EBIAS) must stay < 240 (fp8e4 max):
                 # observed max s*scale is 8.06 on the axon-backend input
                 # draw (6.2 on the CPU draw); -4 leaves >1 margin incl.
                 # fp8 q/k quantization jitter. Normalization cancels it.

_CACHED = {}


def build_nc():
    nc = bacc.Bacc("TRN2", target_bir_lowering=False, debug=False)

    # bundle row c = [x_strip0[c, 0:512] | wv[c, :] | wqk[c, :]] so each kc
    # contraction chunk arrives in ONE 4KB-per-partition DMA (HWDGE is a
    # serial 625ns-per-DMA resource; descriptor splits are what cost time).
    bun_d = nc.dram_tensor("bun", [C, 4 * CL], BF16, kind="ExternalInput")
    # fp8 inputs for the DoubleRow QKV production (strips 1-3)
    x8_d = nc.dram_tensor("x8", [C, T], F8, kind="ExternalInput")
    wqk8_d = nc.dram_tensor("wqk8", [C, 8 * P], F8, kind="ExternalInput")
    wv8_d = nc.dram_tensor("wv8", [C, CL], F8, kind="ExternalInput")
    bqk8_d = nc.dram_tensor("bqk8", [P, 8], F32, kind="ExternalInput")
    wp_d = nc.dram_tensor("wp", [CL, C], BF16, kind="ExternalInput")
    bqk_d = nc.dram_tensor("bqk", [P, 8], F32, kind="ExternalInput")
    tril_d = nc.dram_tensor("tril", [P, 2 * P], BF16, kind="ExternalInput")
    tril8_d = nc.dram_tensor("tril8", [P, 2 * P], F8, kind="ExternalInput")
    e64_d = nc.dram_tensor("e64", [EH, 2], BF16, kind="ExternalInput")
    y_d = nc.dram_tensor("y", [T, C], BF16, kind="ExternalOutput")

    bun_r = bun_d.ap().rearrange("(kc p) c -> p kc c", p=P)     # [128, 8, 2048]
    x8_r = x8_d.ap().rearrange("(kc p) t -> p kc t", p=P)       # [128, 8, 2048]
    wqk8_r = wqk8_d.ap().rearrange("(kc p) c -> p kc c", p=P)   # [128, 8, 1024]
    wv8_r = wv8_d.ap().rearrange("(kc p) c -> p kc c", p=P)     # [128, 8, 512]
    wp_r = wp_d.ap().rearrange("(ct p) c -> p ct c", p=P)       # [128, 4, 1024]
    y_r = y_d.ap().rearrange("(tt p) c -> p tt c", p=P)         # [128, 16, 1024]

    scale = float(HD) ** -0.5
    LAG = 2

    with TileContext(nc) as tc, \
         tc.tile_pool(name="const", bufs=1) as constp, \
         tc.tile_pool(name="big", bufs=1) as bigp, \
         tc.tile_pool(name="u_pool", bufs=int(os.environ.get("K_UBUFS", "12"))) as up, \
         tc.tile_pool(name="norm", bufs=int(os.environ.get("K_NBUFS", "6"))) as normp, \
         tc.tile_pool(name="ystage", bufs=int(os.environ.get("K_YBUFS", "4"))) as ystagep:

        # strip-0 q,k in bf16 c-tile layout (accurate scores for small-n_eff
        # rows); strips 1-3 q,k live in fp8 quadrant layout (q8/k8)
        qk0 = bigp.tile([P, 8, SW], BF16)     # c-tiles 0-3 = qT, 4-7 = kT
        # fp8 quadrant layout: tile X holds heads 4X..4X+3; head h on
        # partitions 32*(h%4)..+32, dim1 = ch-half (2x32), dim2 = t.
        # Feeds 32-partition DoubleRow scores matmuls.
        q8 = [bigp.tile([P, 2, T], F8, name=f"q8_{x}") for x in range(2)]
        k8 = [bigp.tile([P, 2, T], F8, name=f"k8_{x}") for x in range(2)]
        # strip-0 AV runs bf16 (small n_eff rows need accuracy): v chunks 0-3
        v0 = bigp.tile([P, 4, HPC * EH], BF16)
        v0_heads = v0[:].rearrange("p t (h e) -> p t h e", e=EH)
        # strips 1-3 AV runs fp8 DoubleRow: all 16 chunks, ones col at 64
        v8 = bigp.tile([P, NT, HPC * EV], F8)
        v8_heads = v8[:].rearrange("p t (h e) -> p t h e", e=EV)
        attnT = bigp.tile([P, 4, T], BF16)
        bun_sb = bigp.tile([P, KC, 4 * CL], BF16)  # [x0 | wv | wqk] per kc
        x8_sb = bigp.tile([P, KC, T], F8)
        wqk8_sb = bigp.tile([P, KC, 8 * P], F8)
        wv8_sb = bigp.tile([P, KC, CL], F8)
        wp_sb = bigp.tile([P, 4, C], BF16)
        bqk = constp.tile([P, 8], F32)
        bqk8 = constp.tile([P, 8], F32)
        tril = constp.tile([P, 2, P], BF16)
        tril8 = constp.tile([P, 2, P], F8)
        e64 = constp.tile([EH, 2], BF16)
        nbias = constp.tile([P, 1], F32)
        wv_sb = bun_sb[:, :, 0:CL]
        xts0 = bun_sb[:, :, CL:CL + SW]
        wqk_sb = bun_sb[:, :, 2 * CL:4 * CL]

        # -------------------------- input DMAs ---------------------------
        # Strips run in order (1, 2, 3, 0): the fp8 stream goes FIRST so
        # the ACT-saturating fp8 strips start exp'ing ~10us in, and the
        # bf16 strip-0 bundle (consumed last) streams in their shadow.
        # Tiny consts ride the idle Pool SWDGE queue.
        nc.gpsimd.dma_start(bqk[:], bqk_d[:])
        nc.gpsimd.dma_start(bqk8[:], bqk8_d[:])
        nc.gpsimd.dma_start(
            tril[:], tril_d.ap().rearrange("p (h q) -> p h q", h=2))
        nc.gpsimd.dma_start(
            tril8[:], tril8_d.ap().rearrange("p (h q) -> p h q", h=2))
        nc.gpsimd.dma_start(e64[:], e64_d[:])
        nc.gpsimd.memset(nbias[:], EBIAS)
        nc.gpsimd.memset(v0_heads[:, :, :, HD], 1.0)
        nc.gpsimd.memset(v8_heads[:, :, :, HD], 1.0)
        # fp8 stream, ordered by first consumption: wqk8 k-groups (early
        # k0 production), x8 span 0, wqk8 q-groups, x8 span 1, wv8
        # (v8 chunks), x8 spans 2/3
        nc.sync.dma_start(wqk8_sb[:, 0:4, 4 * P:], wqk8_r[:, 0:4, 4 * P:])
        nc.sync.dma_start(wqk8_sb[:, 4:8, 4 * P:], wqk8_r[:, 4:8, 4 * P:])
        nc.sync.dma_start(x8_sb[:, 0:4, 0:SW], x8_r[:, 0:4, 0:SW])
        nc.sync.dma_start(x8_sb[:, 4:8, 0:SW], x8_r[:, 4:8, 0:SW])
        nc.sync.dma_start(wqk8_sb[:, :, 0:4 * P], wqk8_r[:, :, 0:4 * P])
        nc.sync.dma_start(x8_sb[:, 0:4, SW:2 * SW], x8_r[:, 0:4, SW:2 * SW])
        nc.sync.dma_start(x8_sb[:, 4:8, SW:2 * SW], x8_r[:, 4:8, SW:2 * SW])
        nc.sync.dma_start(wv8_sb[:], wv8_r)
        for sp in (2, 3):
            nc.sync.dma_start(x8_sb[:, :, sp * SW:(sp + 1) * SW],
                              x8_r[:, :, sp * SW:(sp + 1) * SW])
        # bf16 bundle for strip-0 production (consumed by strip-2 fillers)
        for kc in range(KC):
            nc.sync.dma_start(bun_sb[:, kc, :], bun_r[:, kc, :])
        nc.sync.dma_start(wp_sb[:], wp_r)

        # warm the exp table during the DMA lead-in (LoadActFuncSet is lazy
        # and otherwise lands on the first-scores critical path)
        warm = constp.tile([1, 2], F32)
        nc.gpsimd.memset(warm[:], 0.0)
        nc.scalar.activation(warm[:], warm[:], EXPF)

        with tc.tile_pool(name="mm", bufs=2, space="PSUM") as mmp, \
             tc.tile_pool(name="ps_s", bufs=2, space="PSUM") as ps_sp, \
             tc.tile_pool(name="ps_o", bufs=2, space="PSUM") as ps_op:

            # ---------------- op generators -------------------------------
            def gen_qk8(s, which, txs=(0, 1), c0=0, c1=SW):
                """fp8 DoubleRow q/k production for strip-span s into the
                quadrant layout (4x contraction rate vs bf16); one yield
                per instruction. c0/c1 select a column sub-span (used to
                bootstrap the first k chunks during the DMA lead-in)."""
                cols = slice(s * SW + c0, s * SW + c1)
                for qk in which:           # 0 = q, 1 = k
                    dst = q8 if qk == 0 else k8
                    for tx in txs:
                        for half in range(2):
                            gi = qk * 4 + tx * 2 + half
                            psq = mmp.tile([P, c1 - c0], F32, tag="mm",
                                           name="psq8")
                            for kcp in range(KC // 2):
                                nc.tensor.matmul(
                                    psq[:],
                                    wqk8_sb[:, 2 * kcp:2 * kcp + 2,
                                            gi * P:(gi + 1) * P],
                                    x8_sb[:, 2 * kcp:2 * kcp + 2, cols],
                                    start=(kcp == 0),
                                    stop=(kcp == KC // 2 - 1),
                                    perf_mode=DR,
                                )
                                yield
                            nc.vector.tensor_scalar_add(
                                dst[tx][:, half, cols], psq[:],
                                bqk8[:, gi:gi + 1])
                            yield

            def gen_v8(tchunks):
                """fp8 DoubleRow v production."""
                for tch in tchunks:
                    psv = mmp.tile([P, CL], F32, tag="mm", name="psv8")
                    for kcp in range(KC // 2):
                        nc.tensor.matmul(
                            psv[:],
                            x8_sb[:, 2 * kcp:2 * kcp + 2,
                                  tch * P:(tch + 1) * P],
                            wv8_sb[:, 2 * kcp:2 * kcp + 2, :],
                            start=(kcp == 0), stop=(kcp == KC // 2 - 1),
                            perf_mode=DR,
                        )
                        yield
                    nc.vector.tensor_copy(
                        v8_heads[:, tch, :, 0:HD],
                        psv[:].rearrange("p (h d) -> p h d", d=HD),
                    )
                    yield

            def gen_qk0(cts):
                """bf16 strip-0 q/k c-tile production from the bundle;
                consumed only by strip 0, which runs last."""
                for ct in cts:
                    psq = mmp.tile([P, SW], F32, tag="mm", name="psq0r")
                    for kc in range(KC):
                        nc.tensor.matmul(
                            psq[:],
                            wqk_sb[:, kc, ct * P:(ct + 1) * P],
                            xts0[:, kc, :],
                            start=(kc == 0), stop=(kc == KC - 1),
                        )
                        yield
                    nc.vector.tensor_scalar_add(
                        qk0[:, ct, :], psq[:], bqk[:, ct:ct + 1])
                    yield

            def gen_v0():
                """bf16 v production for strip-0's chunks 0-3 (accurate
                path for the small-n_eff rows)."""
                for tt in range(4):
                    psv = mmp.tile([P, CL], F32, tag="mm", name="psv0")
                    for kc in range(KC):
                        nc.tensor.matmul(
                            psv[:],
                            xts0[:, kc, tt * P:(tt + 1) * P],
                            wv_sb[:, kc, :],
                            start=(kc == 0), stop=(kc == KC - 1),
                        )
                        yield
                    nc.vector.tensor_copy(
                        v0_heads[:, tt, :, 0:HD],
                        psv[:].rearrange("p (h d) -> p h d", d=HD),
                    )
                    yield

            def gen_proj(s):
                """Output projection for strip s; one yield per matmul.
                Both halves of a t-chunk share one [P, 1024] staging tile so
                each t-chunk costs a single (2KB/descriptor) y DMA."""
                for tt4 in range(SW // P):
                    tt = s * (SW // P) + tt4
                    yt = ystagep.tile([P, C], BF16, tag="yt")
                    for co in range(2):
                        psy = mmp.tile([P, 512], F32, tag="mm", name="psy")
                        for ct in range(4):
                            nc.tensor.matmul(
                                psy[:],
                                attnT[:, ct, tt * P:(tt + 1) * P],
                                wp_sb[:, ct, co * 512:(co + 1) * 512],
                                start=(ct == 0), stop=(ct == 3),
                            )
                            yield
                        nc.vector.tensor_copy(
                            yt[:, co * 512:(co + 1) * 512], psy[:])
                        yield
                    nc.sync.dma_start(y_r[:, tt, :], yt[:])

            class Pacer:
                def __init__(self, gens_counts, reserve=0):
                    self.gens = [g for g, n in gens_counts]
                    self.remaining = sum(n for g, n in gens_counts)
                    self.reserve = reserve

                def pump(self, n):
                    for _ in range(n):
                        while self.gens:
                            try:
                                next(self.gens[0])
                                self.remaining -= 1
                                break
                            except StopIteration:
                                self.gens.pop(0)
                        if not self.gens:
                            self.remaining = 0
                            return

                def auto(self, sites_left):
                    # spread the unreserved remainder over remaining sites
                    free = self.remaining - self.reserve
                    if free <= 0 or sites_left <= 0:
                        return
                    self.pump(-(-free // sites_left))

                def drain(self):
                    while self.gens:
                        self.pump(1)

            BPUMP = int(os.environ.get("K_BPUMP", "7"))
            tail_norm = [None]
            pending_norm = [None]
            LAGS = [int(v) for v in
                    os.environ.get("K_LAGS", "3,3,4,4").split(",")]
            RESV = int(os.environ.get("K_RESV", "20"))

            # ---- early direct production: strip-1's scores inputs ------
            # (k span 0, q span 1, k span 1) in DMA-arrival order; the PE
            # is otherwise idle while the fp8 stream lands.
            for g in (gen_qk8(0, (1,)), gen_qk8(1, (0,)),
                      gen_qk8(1, (1,))):
                for _ in g:
                    pass

            # ---------------- fused attention pipeline --------------------
            # Strip order (1, 2, 0, 3): the fp8 strips keep ACT (exp, the
            # bottleneck engine) saturated from ~10us on; bf16 strip 0,
            # whose bundle arrives last on the DMA queue, slots in third
            # (its production hidden in strip-1/2 fillers), and strip 3
            # stays last so the proj tail interleave applies unchanged.
            LASTS = 3
            for s in (1, 2, 0, 3):
                LAG = LAGS[s]
                gens = []
                if s == 1:
                    gens.append((gen_v8(range(0, 8)), 40))
                    gens.append((gen_qk8(2, (0, 1)), 40))
                    gens.append((gen_v8(range(8, 12)), 20))
                elif s == 2:
                    gens.append((gen_qk0(range(8)), 72))
                    gens.append((gen_v0(), 36))
                elif s == 0:
                    gens.append((gen_qk8(3, (0, 1)), 40))
                else:
                    gens.append((gen_v8(range(12, 16)), 20))
                    for ps_ in (1, 2, 0):
                        gens.append((gen_proj(ps_), 40))
                # the last strip holds back ~20 filler matmuls to cover the
                # tail pair's normalize latency during the final projection
                pacer = Pacer(gens, reserve=RESV if s == LASTS else 0)
                nk = (SW // P) * (s + 1)
                sites = 4 * (nk + LAG)

                for pr in range(4):  # head pair (2pr, 2pr+1)
                    qct, kct = pr, 4 + pr
                    # pair 0 of a strip: nothing is in flight yet — big
                    # pre-pumps would just delay the strip's first scores
                    pacer.pump(int(os.environ.get('K_P0', '2'))
                               if pr == 0 else BPUMP)
                    psoA = psoB = None
                    u_ring = {}
                    for step in range(nk + LAG):
                        if (step == int(os.environ.get('K_FLS', '3'))
                                and pending_norm[0] is not None):
                            # previous pair's normalize extract, deferred
                            # here so its psr matmuls never stall the PE
                            # (the oun copies have long since drained)
                            pending_norm[0]()
                            pending_norm[0] = None
                        if step == LAG:
                            # AV accumulators allocated AFTER the deferred
                            # extract so the ps_o ring never hands the psr
                            # tiles a slot aliasing a live accumulator
                            psoA = ps_op.tile([EH, SW], F32, tag="ps_o",
                                              name="psoA")
                            psoB = ps_op.tile([EH, SW], F32, tag="ps_o",
                                              name="psoB")
                        if step < nk:
                            kt = step
                            # columns < q0 of a diagonal tile are fully
                            # masked: skip them entirely; the [128,128]
                            # block at the diagonal is masked on DVE after
                            # the exp.
                            q0 = max(0, kt * P - s * SW)
                            diag = kt >= (SW // P) * s
                            ps = ps_sp.tile([P, 2, SW], F32, tag="ps_s",
                                            name="ps")
                            if s == 0:
                                u = up.tile([P, 2, SW], BF16, tag="u",
                                            name="u")
                                u_ring[kt] = u
                            elif kt % 2 == 0:
                                # fp8 u for a k-chunk PAIR: [p, head, kt2, q]
                                u8 = up.tile([P, 2, 2, SW], F8, tag="u",
                                             name="u8")
                                u_ring[kt // 2] = u8
                            else:
                                u8 = u_ring[kt // 2]
                                if diag:
                                    # odd diag chunk: cols [q0_even, q0) are
                                    # fully masked but inside the pair's AV
                                    # span; zero them (exp never writes them)
                                    q0e = max(0, (kt - 1) * P - s * SW)
                                    if q0 > q0e:
                                        nc.gpsimd.memset(
                                            u8[:, :, 1, q0e:q0], 0.0)
                            if s == 0:
                                for hh in range(2):
                                    hp = hh * HD
                                    nc.tensor.matmul(
                                        ps[:, hh, q0:SW],
                                        qk0[hp:hp + HD, kct,
                                            kt * P:(kt + 1) * P],
                                        qk0[hp:hp + HD, qct, q0:SW],
                                        start=True, stop=True,
                                    )
                            else:
                                # fp8 DoubleRow scores: head quadrant at
                                # partitions 32*qd, ch split 2x32 as k-tiles
                                tx = pr // 2
                                for hh in range(2):
                                    qd = (2 * pr) % 4 + hh
                                    b0 = 32 * qd
                                    nc.tensor.matmul(
                                        ps[:, hh, q0:SW],
                                        k8[tx][b0:b0 + 32, :,
                                               kt * P:(kt + 1) * P],
                                        q8[tx][b0:b0 + 32, :,
                                               s * SW + q0:(s + 1) * SW],
                                        start=True, stop=True,
                                        perf_mode=DR,
                                        # explicit: base_partition() rejects
                                        # 96 but the ISA allows it
                                        tile_position=(b0, 0),
                                    )
                            if s == 0:
                                nc.scalar.activation(
                                    u[:, :, q0:SW], ps[:, :, q0:SW],
                                    EXPF, scale=scale,
                                )
                                if diag:
                                    nc.vector.tensor_tensor(
                                        u[:, :, q0:q0 + P],
                                        u[:, :, q0:q0 + P],
                                        tril[:], MUL,
                                    )
                            else:
                                j = kt % 2
                                nc.scalar.activation(
                                    u8[:, :, j, q0:SW], ps[:, :, q0:SW],
                                    EXPF, scale=scale, bias=nbias[:],
                                )
                                if diag:
                                    nc.vector.tensor_tensor(
                                        u8[:, :, j, q0:q0 + P],
                                        u8[:, :, j, q0:q0 + P],
                                        tril8[:], MUL,
                                    )
                        if s == 0:
                            if step >= LAG:
                                kt = step - LAG
                                u = u_ring.pop(kt)
                                q0 = max(0, kt * P - s * SW)
                                last = kt == nk - 1
                                nc.tensor.matmul(
                                    psoA[0:EH, q0:SW],
                                    v0[:, kt,
                                       (2 * pr) * EH:(2 * pr + 1) * EH],
                                    u[:, 0, q0:SW],
                                    start=(kt == 0), stop=last,
                                )
                                nc.tensor.matmul(
                                    psoB[0:EH, q0:SW],
                                    v0[:, kt,
                                       (2 * pr + 1) * EH:(2 * pr + 2) * EH],
                                    u[:, 1, q0:SW],
                                    start=(kt == 0), stop=last,
                                )
                        elif step >= LAG and (step - LAG) % 2 == 1:
                            # fp8 DoubleRow AV over the chunk pair
                            # (kt0, kt0+1): contraction 256 at 0.5 cyc/col
                            jp = (step - LAG) // 2
                            u8c = u_ring.pop(jp)
                            kt0 = 2 * jp
                            q0p = max(0, kt0 * P - s * SW)
                            for hh, pso in ((0, psoA), (1, psoB)):
                                hcol = (2 * pr + hh) * EV
                                # single full-span inst per pair: psum
                                # start/stop marking is per-2KB bank, so
                                # region-split start=True insts would wipe
                                # each other's accumulation
                                nc.tensor.matmul(
                                    pso[0:EH, q0p:SW],
                                    v8[:, kt0:kt0 + 2,
                                       hcol:hcol + EH],
                                    u8c[:, hh, :, q0p:SW],
                                    start=(kt0 == 0),
                                    stop=(kt0 == nk - 2),
                                    perf_mode=DR,
                                )
                        sites -= 1
                        pacer.auto(sites)

                    # ---- per-pair normalize ----
                    cols = slice(s * SW, (s + 1) * SW)
                    if s == LASTS and pr == 3:
                        # fully exposed tail pair: only the psum copies are
                        # emitted here; the rest of the chain is interleaved
                        # with the final projection stages below so its PE
                        # ops never block the independent ct0 stage.
                        ounAb = normp.tile([EH, SW], BF16, tag="oun",
                                           name="ounAb")
                        ounBb = normp.tile([EH, SW], BF16, tag="oun",
                                           name="ounBb")
                        nc.scalar.copy(ounAb[:], psoA[:])
                        nc.vector.tensor_copy(ounBb[:], psoB[:])

                        def tail_extract():
                            # sums to partition 0 with tiny matmuls instead
                            # of a DMA hop (saves ~2.5us of chain latency)
                            psrA = ps_op.tile([EH, SW], F32, tag="ps_o",
                                              name="psrA")
                            psrB = ps_op.tile([EH, SW], F32, tag="ps_o",
                                              name="psrB")
                            nc.tensor.matmul(psrA[0:1, :], e64[:, 0:1],
                                             ounAb[:], start=True, stop=True)
                            nc.tensor.matmul(psrB[0:1, :], e64[:, 1:2],
                                             ounBb[:], start=True, stop=True)
                            rcA = normp.tile([1, SW], BF16, tag="rc",
                                             name="rcA")
                            rcB = normp.tile([1, SW], BF16, tag="rc",
                                             name="rcB")
                            with nc.allow_low_precision(
                                    reason="softmax sums are O(100); bf16 "
                                           "recip adds ~0.2% scale error"):
                                nc.vector.reciprocal(rcA[:], psrA[0:1, :])
                                nc.vector.reciprocal(rcB[:], psrB[0:1, :])
                            bcA = normp.tile([HD, SW], BF16, tag="bc",
                                             name="bcA")
                            bcB = normp.tile([HD, SW], BF16, tag="bc",
                                             name="bcB")
                            nc.gpsimd.partition_broadcast(bcA[:], rcA[:])
                            nc.gpsimd.partition_broadcast(bcB[:], rcB[:])
                            nc.vector.tensor_tensor(
                                attnT[0:HD, pr, cols], ounAb[0:HD, :],
                                bcA[:], MUL)
                            nc.vector.tensor_tensor(
                                attnT[HD:P, pr, cols], ounBb[0:HD, :],
                                bcB[:], MUL)

                        tail_norm[0] = tail_extract
                        continue
                    # steady state: copy psum out NOW (frees the AV banks
                    # for the next pair); the rest of the chain — e64
                    # matmul sum-extract (no DMA-hop latency), reciprocal,
                    # row broadcast, multiply — is DEFERRED past the next
                    # pair's first scores so the in-order PE never stalls
                    # on the DVE copies. bf16 operands give DVE 2x modes.
                    ounA = normp.tile([EH, SW], BF16, tag="oun", name="ounA")
                    ounB = normp.tile([EH, SW], BF16, tag="oun", name="ounB")
                    nc.vector.tensor_copy(ounA[:], psoA[:])
                    nc.vector.tensor_copy(ounB[:], psoB[:])

                    def steady_extract(pr=pr, cols=cols, ounA=ounA,
                                       ounB=ounB):
                        psrA = ps_op.tile([EH, SW], F32, tag="ps_o",
                                          name="psrA")
                        psrB = ps_op.tile([EH, SW], F32, tag="ps_o",
                                          name="psrB")
                        nc.tensor.matmul(psrA[0:1, :], e64[:, 0:1],
                                         ounA[:], start=True, stop=True)
                        nc.tensor.matmul(psrB[0:1, :], e64[:, 1:2],
                                         ounB[:], start=True, stop=True)
                        rcA = normp.tile([1, SW], BF16, tag="rc",
                                         name="rcA")
                        rcB = normp.tile([1, SW], BF16, tag="rc",
                                         name="rcB")
                        with nc.allow_low_precision(
                                reason="softmax sums are O(100); bf16 "
                                       "recip adds ~0.2% scale error"):
                            nc.vector.reciprocal(rcA[:], psrA[0:1, :])
                            nc.vector.reciprocal(rcB[:], psrB[0:1, :])
                        bcA = normp.tile([HD, SW], BF16, tag="bc",
                                         name="bcA")
                        bcB = normp.tile([HD, SW], BF16, tag="bc",
                                         name="bcB")
                        nc.gpsimd.partition_broadcast(bcA[:], rcA[:])
                        nc.gpsimd.partition_broadcast(bcB[:], rcB[:])
                        nc.vector.tensor_tensor(
                            attnT[0:HD, pr, cols], ounA[0:HD, :],
                            bcA[:], MUL)
                        nc.vector.tensor_tensor(
                            attnT[HD:P, pr, cols], ounB[0:HD, :],
                            bcB[:], MUL)

                    pending_norm[0] = steady_extract

                if s != LASTS:
                    pacer.drain()
                else:
                    tail_pacer = pacer

            # ------------- tail: strip 3 projection -----------------------
            # Stay inside the shared pools (opening a new psum pool is a
            # full drain barrier): six concurrent chains — two [P,512] in
            # mm, plus both co-halves packed into each [P,2,512] score
            # tile — run stage-major so everything except the ct=3 stage
            # overlaps the tail pair's normalize chain; the last two
            # chains follow.
            tts = list(range(4 * LASTS, 4 * LASTS + 4))
            chains = []   # (tt, co, psum_ap)
            for i in range(2):
                ps6 = ps_sp.tile([P, 2, SW], F32, tag="ps_s",
                                 name=f"psf6_{i}")
                chains.append((tts[i], 0, ps6[:, 0, :]))
                chains.append((tts[i], 1, ps6[:, 1, :]))

            def proj_stage(chain_list, ct):
                for tt, co, psy in chain_list:
                    nc.tensor.matmul(
                        psy,
                        attnT[:, ct, tt * P:(tt + 1) * P],
                        wp_sb[:, ct, co * 512:(co + 1) * 512],
                        start=(ct == 0), stop=(ct == 3),
                    )

            proj_stage(chains, 0)
            tail_norm[0]()   # extract/recip/broadcast/mult, off-PE mostly
            proj_stage(chains, 1)
            proj_stage(chains, 2)
            # held-back fillers bridge the normalize chain; they rotate the
            # mm slots, so the mm-hosted tail chains allocate only after.
            tail_pacer.drain()
            # tt15 in the AV banks (free after the sum-extract recips),
            # tt14 in mm; their ct0-2 stages also cover the chain latency.
            chains_o = []
            for i in range(2):
                pso6 = ps_op.tile([P, SW], F32, tag="ps_o", name=f"psfo_{i}")
                chains_o.append((tts[3], i, pso6[:]))
            chains_mm = []
            for i in range(2):
                psm = mmp.tile([P, 512], F32, tag="mm", name=f"psf2_{i}")
                chains_mm.append((tts[2], i, psm[:]))
            for ct in range(3):
                proj_stage(chains_o, ct)
                proj_stage(chains_mm, ct)
            yts = {}

            def proj_drain(tt, co, psy, i):
                if tt not in yts:
                    yts[tt] = ystagep.tile([P, C], BF16, tag="yt",
                                           name=f"ytf_{tt}")
                yt = yts[tt]
                if i % 2 == 0:
                    nc.scalar.copy(yt[:, co * 512:(co + 1) * 512], psy)
                else:
                    nc.vector.tensor_copy(
                        yt[:, co * 512:(co + 1) * 512], psy)
                if co == 1:
                    nc.sync.dma_start(y_r[:, tt, :], yt[:])

            proj_stage(chains, 3)
            # tts 12/13 live in single [P,2,512] tiles: one wide copy each
            # (ACT and DVE in parallel), DMA as soon as each lands
            for i in range(2):
                yt = ystagep.tile([P, C], BF16, tag="yt",
                                  name=f"ytf_{tts[i]}")
                ytv = yt[:].rearrange("p (a c) -> p a c", a=2)
                src = chains[2 * i][2].tensor.ap()
                if i == 0:
                    nc.scalar.copy(ytv, src)
                else:
                    nc.vector.tensor_copy(ytv, src)
                nc.sync.dma_start(y_r[:, tts[i], :], yt[:])
            proj_stage(chains_o, 3)
            proj_stage(chains_mm, 3)
            for i, (tt, co, psy) in enumerate(chains_o + chains_mm):
                proj_drain(tt, co, psy, i)
    nc.compile()
    return nc


def _host_consts():
    import ml_dtypes
    i_idx = np.arange(P, dtype=np.float32)[:, None]
    j_idx = np.arange(P, dtype=np.float32)[None, :]
    trf = (j_idx - i_idx >= 0).astype(np.float32)         # [k, q]: keep k<=q
    tr = trf.astype(ml_dtypes.bfloat16)
    tril = np.concatenate([tr, tr], axis=1)               # [P, 2*P]
    tr8 = trf.astype(ml_dtypes.float8_e4m3)
    tril8 = np.concatenate([tr8, tr8], axis=1)
    e64 = np.zeros((EH, 2), dtype=ml_dtypes.bfloat16)
    e64[HD, :] = 1
    return tril, tril8, e64


def make_in_maps(x, w_attn, b_attn, w_proj):
    import ml_dtypes
    bf = ml_dtypes.bfloat16
    f8 = ml_dtypes.float8_e4m3
    tril, tril8, e64 = _host_consts()
    # fp8 quadrant column permutation: production group gi = qk*4+tx*2+half,
    # col j -> local head tx*4 + j//32, channel (j%32) + 32*half
    j = np.arange(P)
    gidx = np.empty((8, P), dtype=np.int64)
    for gi in range(8):
        qk, tx, half = gi // 4, (gi // 2) % 2, gi % 2
        lh = tx * 4 + j // 32
        ch = (j % 32) + 32 * half
        gidx[gi] = qk * C + lh * HD + ch
    in_maps = []
    for c in range(8):
        b = c // 2
        h0 = (c % 2) * HPC
        qcols = slice(h0 * HD, h0 * HD + CL)
        kcols = slice(C + h0 * HD, C + h0 * HD + CL)
        vcols = slice(2 * C + h0 * HD, 2 * C + h0 * HD + CL)
        xt = np.ascontiguousarray(x[b].T)
        wqk = np.concatenate(
            [w_attn[:, qcols], w_attn[:, kcols]], axis=1).astype(bf)
        wv = w_attn[:, vcols].astype(bf)
        bun = np.concatenate([wv, xt[:, 0:SW].astype(bf), wqk], axis=1)
        bqk = np.concatenate([b_attn[qcols], b_attn[kcols]]).reshape(8, P).T
        cidx = (gidx + h0 * HD).reshape(-1)       # [8*128] global w cols
        wqk8 = w_attn[:, cidx].astype(f8)
        bqk8 = b_attn[cidx].reshape(8, P).T.astype(np.float32)
        in_maps.append({
            "bun": np.ascontiguousarray(bun),
            "x8": xt.astype(f8),
            "wqk8": np.ascontiguousarray(wqk8),
            "wv8": np.ascontiguousarray(w_attn[:, vcols]).astype(f8),
            "bqk8": np.ascontiguousarray(bqk8),
            "wp": np.ascontiguousarray(
                w_proj[h0 * HD:h0 * HD + CL, :]).astype(bf),
            "bqk": np.ascontiguousarray(bqk),
            "tril": tril,
            "tril8": tril8,
            "e64": e64,
        })
    return in_maps


def _get_runner():
    """Build the SPMD executor once: a cached jax.jit over 8 cores.

    Mirrors bass2jax.run_bass_via_pjrt but hoists the jit so repeated
    kernel() calls reuse the compiled executable.
    """
    if "runner" in _CACHED:
        return _CACHED["runner"]
    import jax
    import jax.numpy as jnp
    from jax.sharding import Mesh, PartitionSpec
    from jax.experimental.shard_map import shard_map
    from concourse import bass2jax
    import concourse.mybir as mybir_

    nc = _CACHED.get("nc")
    if nc is None:
        nc = _CACHED["nc"] = build_nc()
    bass2jax.install_neuronx_cc_hook()

    partition_name = (nc.partition_id_tensor.name
                      if nc.partition_id_tensor else None)
    in_names, out_names, out_avals, zero_shapes = [], [], [], []
    for alloc in nc.m.functions[0].allocations:
        if not isinstance(alloc, mybir_.MemoryLocationSet):
            continue
        name = alloc.memorylocations[0].name
        if alloc.kind == "ExternalInput":
            if name != partition_name:
                in_names.append(name)
        elif alloc.kind == "ExternalOutput":
            shape = tuple(alloc.tensor_shape)
            dtype = mybir_.dt.np(alloc.dtype)
            out_names.append(name)
            out_avals.append(jax.core.ShapedArray(shape, dtype))
            zero_shapes.append((shape, dtype))
    n_params = len(in_names)
    n_outs = len(out_names)
    all_names = in_names + out_names
    if partition_name is not None:
        all_names = all_names + [partition_name]

    def _body(*args):
        operands = list(args)
        if partition_name is not None:
            operands.append(bass2jax.partition_id_tensor())
        outs = bass2jax._bass_exec_p.bind(
            *operands,
            out_avals=tuple(out_avals),
            in_names=tuple(all_names),
            out_names=tuple(out_names),
            lowering_input_output_aliases=(),
            sim_require_finite=True,
            sim_require_nnan=True,
            nc=nc,
        )
        return tuple(outs)

    devices = jax.devices()[:8]
    mesh = Mesh(np.asarray(devices), ("core",))
    in_specs = (PartitionSpec("core"),) * (n_params + n_outs)
    out_specs = (PartitionSpec("core"),) * n_outs
    donate = tuple(range(n_params, n_params + n_outs))
    sharded = jax.jit(
        shard_map(_body, mesh=mesh, in_specs=in_specs, out_specs=out_specs,
                  check_rep=False),
        donate_argnums=donate, keep_unused=True,
    )

    def run(in_maps):
        concat_in = [
            np.concatenate([np.asarray(in_maps[c][nm]) for c in range(8)],
                           axis=0)
            for nm in in_names
        ]
        concat_zeros = [
            np.zeros((8 * s[0], *s[1:]), dt) for (s, dt) in zero_shapes
        ]
        out_arrs = sharded(*concat_in, *concat_zeros)
        return [
            {nm: np.asarray(out_arrs[i]).reshape(8, *out_avals[i].shape)[c]
             for i, nm in enumerate(out_names)}
            for c in range(8)
        ]

    _CACHED["runner"] = run
    return run


def kernel(x, w_attn, b_attn, w_proj, b_proj):
    x = np.asarray(x, dtype=np.float32)
    w_attn = np.asarray(w_attn, dtype=np.float32)
    b_attn = np.asarray(b_attn, dtype=np.float32)
    w_proj = np.asarray(w_proj, dtype=np.float32)
    b_proj = np.asarray(b_proj, dtype=np.float32)

    in_maps = make_in_maps(x, w_attn, b_attn, w_proj)
    results = None
    try:
        run = _get_runner()
        # The first (cold) execution occasionally races on input
        # streaming and corrupts one core's output (sometimes NaN,
        # sometimes silently). Clean executions are bit-deterministic,
        # so run twice and accept only a matching pair; tie-break with
        # extra runs. Device time per run is ~180us, so this is cheap.
        def _ys(r):
            return np.stack([c["y"].astype(np.float32) for c in r])

        prev = None
        for _ in range(4):
            cur = run(in_maps)
            ycur = _ys(cur)
            if not np.isfinite(ycur).all():
                continue
            if prev is not None and np.array_equal(prev[1], ycur):
                results = cur
                break
            prev = (cur, ycur)
        if results is None and prev is not None:
            results = prev[0]
    except Exception:
        results = None
    if results is None:
        # fallback: the stock SPMD runner (slower per call, same result)
        if "nc" not in _CACHED:
            _CACHED["nc"] = build_nc()
        res = run_bass_kernel_spmd(
            _CACHED["nc"], in_maps, core_ids=list(range(8)))
        results = res.results

    # v-bias contribution: probs rows sum to 1, so attn += 1 * b_v^T, and
    # (1 b_v^T) @ w_proj = row vector b_v @ w_proj added to every position.
    extra = b_attn[2 * C:] @ w_proj + b_proj  # [C]
    out = np.empty((B, T, C), dtype=np.float32)
    for b in range(B):
        out[b] = (results[2 * b]["y"].astype(np.float32)
                  + results[2 * b + 1]["y"].astype(np.float32) + extra)
    return out



# revision 125
# speedup vs baseline: 1.0002x; 1.0002x over previous
"""Causal multi-head attention (B=4, T=2048, C=1024, H=16) on 8 Trainium2 cores.

Sharding: core c handles batch b = c//2 and heads h0..h0+7 with h0 = (c%2)*8.
Each core computes QKV projection for its head slice, causal attention for its
8 heads, and a partial output projection. Host sums the two partials per batch
and adds the bias terms.

fp8e4 DoubleRow pipeline (180us HW; bf16 baseline was 237us at 94% PE):
  - fp8e4 matmuls with MatmulPerfMode.DoubleRow process 2 contraction
    k-tiles per instruction at 0.5 cycles/column -- 4x bf16 FLOP rate
    when the contraction fills 256, 2x when it is only 64 (scores).
    QKV production, scores, and AV for strips 1-3 all run this way; the
    PE drops from 226us busy to ~124us and the Activation engine's exp
    stream (~150us busy, element count fixed by the causal mask and
    psum-bank-limited instruction sizes) becomes the binding engine.
    Layouts: q8/k8 hold each head's 64 channels as 2x32 "k-tiles" on one
    partition quadrant (32-row matmuls at tile_position 0/32/64/96); v8
    packs [t-chunk, head*68] so AV pairs adjacent k-chunks as k-tiles,
    with the softmax ones-column riding at slot 64 (M=65 out rows).
  - Accuracy: rows with few attended keys cannot average away fp8 noise,
    so strip 0 (q in [0,512), n_eff as low as 1) keeps the full bf16
    path: bf16 q/k c-tiles, bf16 v chunks 0-3, bf16 u. Strips 1-3
    (n_eff >= ~190) run raw fp8: max-abs rel err 9.3e-3 vs the 2e-2
    gate (bf16 baseline was 4.1e-3). exp writes u as fp8 with bias -2
    so e^s stays below fp8e4's 240 max; normalization cancels the bias.
  - Strip order (1, 2, 0, 3): the fp8 stream (wqk8, x8, wv8) is DMA'd
    first and strip-1 scores start ~10us in, keeping ACT saturated from
    then on; the bf16 bundle for strip 0 streams in the fp8 strips'
    shadow and its production runs as strip-1/2 filler matmuls. Strip 3
    stays last: its long exp tail hides all deferred projection work.
  - Per-pair softmax normalize: psum -> bf16 oun copies (frees the AV
    banks), then a DEFERRED chain -- e64-matmul sum extract (no DMA
    hop), bf16 reciprocal + row broadcast + multiply (DVE 2x modes) --
    emitted after the NEXT pair's first scores so the in-order PE never
    stalls behind DVE queueing at pair boundaries.
  - One psum pool set for the whole program (pool open/close is a full
    drain barrier): 2x scores [P,2,512] (4 banks) + 2x AV [65,512]
    (2 banks, rotating with the deferred sum-extract tiles) + 2 filler
    banks. The projection tail stage-majors across the same banks with
    held-back fillers bridging the tail pair's normalize latency.
  - y is stored bf16 (halves output DMA); host sums partials in f32 and
    folds the v-bias via b_v @ w_proj (softmax rows sum to 1).
"""

import os
import sys
import numpy as np

sys.path.insert(0, "/opt/trn_rl_repo")

import concourse.bass as bass  # noqa: E402
import concourse.bacc as bacc  # noqa: E402
import concourse.mybir as mybir  # noqa: E402
from concourse.bass_utils import run_bass_kernel_spmd  # noqa: E402
from concourse.tile import TileContext  # noqa: E402

B, T, C, H = 4, 2048, 1024, 16
HD = C // H          # 64 head dim
HPC = 8              # heads per core
P = 128
NT = T // P          # 16 t-chunks of 128
SW = 512             # strip width (q and t strips)
NS = T // SW         # 4 strips
KC = C // P          # 8 contraction chunks for QKV
CL = HPC * HD        # 512 local channels per section
EH = HD + 1          # 65: head slot width in v (value cols + ones col)
F32 = mybir.dt.float32
BF16 = mybir.dt.bfloat16
F8 = mybir.dt.float8e4
DR = mybir.MatmulPerfMode.DoubleRow
EV = 68          # v8 head slot: 64 v cols + ones col at 64 + 3 pad (4B align)
EXPF = mybir.ActivationFunctionType.Exp
MUL = mybir.AluOpType.mult
DIV = mybir.AluOpType.divide
EBIAS = -4.0     # exp(s*scale + EBIAS) must stay < 240 (fp8e4 max):
                 # observed max s*scale is 8.06 on the axon-backend input
                 # draw (6.2 on the CPU draw); -4 leaves >1 margin incl.
                 # fp8 q/k quantization jitter. Normalization cancels it.

_CACHED = {}


def build_nc():
    nc = bacc.Bacc("TRN2", target_bir_lowering=False, debug=False)

    # bundle row c = [x_strip0[c, 0:512] | wv[c, :] | wqk[c, :]] so each kc
    # contraction chunk arrives in ONE 4KB-per-partition DMA (HWDGE is a
    # serial 625ns-per-DMA resource; descriptor splits are what cost time).
    bun_d = nc.dram_tensor("bun", [C, 4 * CL], BF16, kind="ExternalInput")
    # fp8 inputs for the DoubleRow QKV production (strips 1-3)
    x8_d = nc.dram_tensor("x8", [C, T], F8, kind="ExternalInput")
    wqk8_d = nc.dram_tensor("wqk8", [C, 8 * P], F8, kind="ExternalInput")
    wv8_d = nc.dram_tensor("wv8", [C, CL], F8, kind="ExternalInput")
    bqk8_d = nc.dram_tensor("bqk8", [P, 8], F32, kind="ExternalInput")
    wp_d = nc.dram_tensor("wp", [CL, C], BF16, kind="ExternalInput")
    bqk_d = nc.dram_tensor("bqk", [P, 8], F32, kind="ExternalInput")
    tril_d = nc.dram_tensor("tril", [P, 2 * P], BF16, kind="ExternalInput")
    tril8_d = nc.dram_tensor("tril8", [P, 2 * P], F8, kind="ExternalInput")
    e64_d = nc.dram_tensor("e64", [EH, 2], BF16, kind="ExternalInput")
    y_d = nc.dram_tensor("y", [T, C], BF16, kind="ExternalOutput")

    bun_r = bun_d.ap().rearrange("(kc p) c -> p kc c", p=P)     # [128, 8, 2048]
    x8_r = x8_d.ap().rearrange("(kc p) t -> p kc t", p=P)       # [128, 8, 2048]
    wqk8_r = wqk8_d.ap().rearrange("(kc p) c -> p kc c", p=P)   # [128, 8, 1024]
    wv8_r = wv8_d.ap().rearrange("(kc p) c -> p kc c", p=P)     # [128, 8, 512]
    wp_r = wp_d.ap().rearrange("(ct p) c -> p ct c", p=P)       # [128, 4, 1024]
    y_r = y_d.ap().rearrange("(tt p) c -> p tt c", p=P)         # [128, 16, 1024]

    scale = float(HD) ** -0.5
    LAG = 2

    with TileContext(nc) as tc, \
         tc.tile_pool(name="const", bufs=1) as constp, \
         tc.tile_pool(name="big", bufs=1) as bigp, \
         tc.tile_pool(name="u_pool", bufs=int(os.environ.get("K_UBUFS", "12"))) as up, \
         tc.tile_pool(name="norm", bufs=int(os.environ.get("K_NBUFS", "6"))) as normp, \
         tc.tile_pool(name="ystage", bufs=int(os.environ.get("K_YBUFS", "4"))) as ystagep:

        # strip-0 q,k in bf16 c-tile layout (accurate scores for small-n_eff
        # rows); strips 1-3 q,k live in fp8 quadrant layout (q8/k8)
        qk0 = bigp.tile([P, 8, SW], BF16)     # c-tiles 0-3 = qT, 4-7 = kT
        # fp8 quadrant layout: tile X holds heads 4X..4X+3; head h on
        # partitions 32*(h%4)..+32, dim1 = ch-half (2x32), dim2 = t.
        # Feeds 32-partition DoubleRow scores matmuls.
        q8 = [bigp.tile([P, 2, T], F8, name=f"q8_{x}") for x in range(2)]
        k8 = [bigp.tile([P, 2, T], F8, name=f"k8_{x}") for x in range(2)]
        # strip-0 AV runs bf16 (small n_eff rows need accuracy): v chunks 0-3
        v0 = bigp.tile([P, 4, HPC * EH], BF16)
        v0_heads = v0[:].rearrange("p t (h e) -> p t h e", e=EH)
        # strips 1-3 AV runs fp8 DoubleRow: all 16 chunks, ones col at 64
        v8 = bigp.tile([P, NT, HPC * EV], F8)
        v8_heads = v8[:].rearrange("p t (h e) -> p t h e", e=EV)
        attnT = bigp.tile([P, 4, T], BF16)
        bun_sb = bigp.tile([P, KC, 4 * CL], BF16)  # [x0 | wv | wqk] per kc
        x8_sb = bigp.tile([P, KC, T], F8)
        wqk8_sb = bigp.tile([P, KC, 8 * P], F8)
        wv8_sb = bigp.tile([P, KC, CL], F8)
        wp_sb = bigp.tile([P, 4, C], BF16)
        bqk = constp.tile([P, 8], F32)
        bqk8 = constp.tile([P, 8], F32)
        tril = constp.tile([P, 2, P], BF16)
        tril8 = constp.tile([P, 2, P], F8)
        e64 = constp.tile([EH, 2], BF16)
        nbias = constp.tile([P, 1], F32)
        wv_sb = bun_sb[:, :, 0:CL]
        xts0 = bun_sb[:, :, CL:CL + SW]
        wqk_sb = bun_sb[:, :, 2 * CL:4 * CL]

        # -------------------------- input DMAs ---------------------------
        # Strips run in order (1, 2, 3, 0): the fp8 stream goes FIRST so
        # the ACT-saturating fp8 strips start exp'ing ~10us in, and the
        # bf16 strip-0 bundle (consumed last) streams in their shadow.
        # Tiny consts ride the idle Pool SWDGE queue; the wtiny memset
        # leads so the p-state warm-up chain can start immediately.
        wtiny = constp.tile([2, SW], BF16)
        nc.gpsimd.memset(wtiny[:], 0.0)
        nc.gpsimd.dma_start(bqk[:], bqk_d[:])
        nc.gpsimd.dma_start(bqk8[:], bqk8_d[:])
        nc.gpsimd.dma_start(
            tril[:], tril_d.ap().rearrange("p (h q) -> p h q", h=2))
        nc.gpsimd.dma_start(
            tril8[:], tril8_d.ap().rearrange("p (h q) -> p h q", h=2))
        nc.gpsimd.dma_start(e64[:], e64_d[:])
        nc.gpsimd.memset(nbias[:], EBIAS)
        nc.gpsimd.memset(v0_heads[:, :, :, HD], 1.0)
        nc.gpsimd.memset(v8_heads[:, :, :, HD], 1.0)
        # fp8 stream, ordered by first consumption: wqk8 k-groups (early
        # k0 production), x8 span 0, wqk8 q-groups, x8 span 1, wv8
        # (v8 chunks), x8 spans 2/3
        nc.sync.dma_start(wqk8_sb[:, 0:4, 4 * P:], wqk8_r[:, 0:4, 4 * P:])
        nc.sync.dma_start(wqk8_sb[:, 4:8, 4 * P:], wqk8_r[:, 4:8, 4 * P:])
        nc.sync.dma_start(x8_sb[:, 0:4, 0:SW], x8_r[:, 0:4, 0:SW])
        nc.sync.dma_start(x8_sb[:, 4:8, 0:SW], x8_r[:, 4:8, 0:SW])
        nc.sync.dma_start(wqk8_sb[:, :, 0:4 * P], wqk8_r[:, :, 0:4 * P])
        nc.sync.dma_start(x8_sb[:, 0:4, SW:2 * SW], x8_r[:, 0:4, SW:2 * SW])
        nc.sync.dma_start(x8_sb[:, 4:8, SW:2 * SW], x8_r[:, 4:8, SW:2 * SW])
        nc.sync.dma_start(wv8_sb[:], wv8_r)
        for sp in (2, 3):
            nc.sync.dma_start(x8_sb[:, :, sp * SW:(sp + 1) * SW],
                              x8_r[:, :, sp * SW:(sp + 1) * SW])
        # bf16 bundle for strip-0 production (consumed by strip-2 fillers)
        for kc in range(KC):
            nc.sync.dma_start(bun_sb[:, kc, :], bun_r[:, kc, :])
        nc.sync.dma_start(wp_sb[:], wp_r)

        # warm the exp table during the DMA lead-in (LoadActFuncSet is lazy
        # and otherwise lands on the first-scores critical path)
        warm = constp.tile([1, 2], F32)
        nc.gpsimd.memset(warm[:], 0.0)
        nc.scalar.activation(warm[:], warm[:], EXPF)

        with tc.tile_pool(name="mm", bufs=2, space="PSUM") as mmp, \
             tc.tile_pool(name="ps_s", bufs=2, space="PSUM") as ps_sp, \
             tc.tile_pool(name="ps_o", bufs=2, space="PSUM") as ps_op:

            # ---------------- op generators -------------------------------
            def gen_qk8(s, which, txs=(0, 1), c0=0, c1=SW):
                """fp8 DoubleRow q/k production for strip-span s into the
                quadrant layout (4x contraction rate vs bf16); one yield
                per instruction. c0/c1 select a column sub-span (used to
                bootstrap the first k chunks during the DMA lead-in)."""
                cols = slice(s * SW + c0, s * SW + c1)
                for qk in which:           # 0 = q, 1 = k
                    dst = q8 if qk == 0 else k8
                    for tx in txs:
                        for half in range(2):
                            gi = qk * 4 + tx * 2 + half
                            psq = mmp.tile([P, c1 - c0], F32, tag="mm",
                                           name="psq8")
                            for kcp in range(KC // 2):
                                nc.tensor.matmul(
                                    psq[:],
                                    wqk8_sb[:, 2 * kcp:2 * kcp + 2,
                                            gi * P:(gi + 1) * P],
                                    x8_sb[:, 2 * kcp:2 * kcp + 2, cols],
                                    start=(kcp == 0),
                                    stop=(kcp == KC // 2 - 1),
                                    perf_mode=DR,
                                )
                                yield
                            nc.vector.tensor_scalar_add(
                                dst[tx][:, half, cols], psq[:],
                                bqk8[:, gi:gi + 1])
                            yield

            def gen_v8(tchunks):
                """fp8 DoubleRow v production."""
                for tch in tchunks:
                    psv = mmp.tile([P, CL], F32, tag="mm", name="psv8")
                    for kcp in range(KC // 2):
                        nc.tensor.matmul(
                            psv[:],
                            x8_sb[:, 2 * kcp:2 * kcp + 2,
                                  tch * P:(tch + 1) * P],
                            wv8_sb[:, 2 * kcp:2 * kcp + 2, :],
                            start=(kcp == 0), stop=(kcp == KC // 2 - 1),
                            perf_mode=DR,
                        )
                        yield
                    nc.vector.tensor_copy(
                        v8_heads[:, tch, :, 0:HD],
                        psv[:].rearrange("p (h d) -> p h d", d=HD),
                    )
                    yield

            def gen_qk0(cts):
                """bf16 strip-0 q/k c-tile production from the bundle;
                consumed only by strip 0, which runs last."""
                for ct in cts:
                    psq = mmp.tile([P, SW], F32, tag="mm", name="psq0r")
                    for kc in range(KC):
                        nc.tensor.matmul(
                            psq[:],
                            wqk_sb[:, kc, ct * P:(ct + 1) * P],
                            xts0[:, kc, :],
                            start=(kc == 0), stop=(kc == KC - 1),
                        )
                        yield
                    nc.vector.tensor_scalar_add(
                        qk0[:, ct, :], psq[:], bqk[:, ct:ct + 1])
                    yield

            def gen_v0():
                """bf16 v production for strip-0's chunks 0-3 (accurate
                path for the small-n_eff rows)."""
                for tt in range(4):
                    psv = mmp.tile([P, CL], F32, tag="mm", name="psv0")
                    for kc in range(KC):
                        nc.tensor.matmul(
                            psv[:],
                            xts0[:, kc, tt * P:(tt + 1) * P],
                            wv_sb[:, kc, :],
                            start=(kc == 0), stop=(kc == KC - 1),
                        )
                        yield
                    nc.vector.tensor_copy(
                        v0_heads[:, tt, :, 0:HD],
                        psv[:].rearrange("p (h d) -> p h d", d=HD),
                    )
                    yield

            def gen_proj(s):
                """Output projection for strip s; one yield per matmul.
                Both halves of a t-chunk share one [P, 1024] staging tile so
                each t-chunk costs a single (2KB/descriptor) y DMA."""
                for tt4 in range(SW // P):
                    tt = s * (SW // P) + tt4
                    yt = ystagep.tile([P, C], BF16, tag="yt")
                    for co in range(2):
                        psy = mmp.tile([P, 512], F32, tag="mm", name="psy")
                        for ct in range(4):
                            nc.tensor.matmul(
                                psy[:],
                                attnT[:, ct, tt * P:(tt + 1) * P],
                                wp_sb[:, ct, co * 512:(co + 1) * 512],
                                start=(ct == 0), stop=(ct == 3),
                            )
                            yield
                        nc.vector.tensor_copy(
                            yt[:, co * 512:(co + 1) * 512], psy[:])
                        yield
                    nc.sync.dma_start(y_r[:, tt, :], yt[:])

            class Pacer:
                def __init__(self, gens_counts, reserve=0):
                    self.gens = [g for g, n in gens_counts]
                    self.remaining = sum(n for g, n in gens_counts)
                    self.reserve = reserve

                def pump(self, n):
                    for _ in range(n):
                        while self.gens:
                            try:
                                next(self.gens[0])
                                self.remaining -= 1
                                break
                            except StopIteration:
                                self.gens.pop(0)
                        if not self.gens:
                            self.remaining = 0
                            return

                def auto(self, sites_left):
                    # spread the unreserved remainder over remaining sites
                    free = self.remaining - self.reserve
                    if free <= 0 or sites_left <= 0:
                        return
                    self.pump(-(-free // sites_left))

                def drain(self):
                    while self.gens:
                        self.pump(1)

            BPUMP = int(os.environ.get("K_BPUMP", "7"))
            tail_norm = [None]
            pending_norm = [None]
            LAGS = [int(v) for v in
                    os.environ.get("K_LAGS", "3,3,4,4").split(",")]
            RESV = int(os.environ.get("K_RESV", "20"))

            # PE p-state warm-up: dummy matmuls run contiguously INTO the
            # first production chain so the 3us ramp to 2.4GHz completes
            # and persists (idle resets it)
            NW = int(os.environ.get("K_NW", "0"))
            if NW:
                pswm = mmp.tile([2, SW], F32, tag="mm", name="pswm")
                for _ in range(NW):
                    nc.tensor.matmul(pswm[0:2, :], wtiny[:, 0:2], wtiny[:],
                                     start=True, stop=True)

            # ---- early direct production: strip-1's scores inputs ------
            # (k span 0, q span 1, k span 1) in DMA-arrival order; the PE
            # is otherwise idle while the fp8 stream lands.
            for g in (gen_qk8(0, (1,)), gen_qk8(1, (0,)),
                      gen_qk8(1, (1,))):
                for _ in g:
                    pass

            # ---------------- fused attention pipeline --------------------
            # Strip order (1, 2, 0, 3): the fp8 strips keep ACT (exp, the
            # bottleneck engine) saturated from ~10us on; bf16 strip 0,
            # whose bundle arrives last on the DMA queue, slots in third
            # (its production hidden in strip-1/2 fillers), and strip 3
            # stays last so the proj tail interleave applies unchanged.
            LASTS = 3
            for s in (1, 2, 0, 3):
                LAG = LAGS[s]
                gens = []
                if s == 1:
                    gens.append((gen_v8(range(0, 8)), 40))
                    gens.append((gen_qk8(2, (0, 1)), 40))
                    gens.append((gen_v8(range(8, 12)), 20))
                elif s == 2:
                    gens.append((gen_qk0(range(8)), 72))
                    gens.append((gen_v0(), 36))
                elif s == 0:
                    gens.append((gen_qk8(3, (0, 1)), 40))
                else:
                    gens.append((gen_v8(range(12, 16)), 20))
                    for ps_ in (1, 2, 0):
                        gens.append((gen_proj(ps_), 40))
                # the last strip holds back ~20 filler matmuls to cover the
                # tail pair's normalize latency during the final projection
                pacer = Pacer(gens, reserve=RESV if s == LASTS else 0)
                nk = (SW // P) * (s + 1)
                sites = 4 * (nk + LAG)

                for pr in range(4):  # head pair (2pr, 2pr+1)
                    qct, kct = pr, 4 + pr
                    # pair 0 of a strip: nothing is in flight yet — big
                    # pre-pumps would just delay the strip's first scores
                    pacer.pump(int(os.environ.get('K_P0', '2'))
                               if pr == 0 else BPUMP)
                    psoA = psoB = None
                    u_ring = {}
                    for step in range(nk + LAG):
                        if (step == int(os.environ.get('K_FLS', '3'))
                                and pending_norm[0] is not None):
                            # previous pair's normalize extract, deferred
                            # here so its psr matmuls never stall the PE
                            # (the oun copies have long since drained)
                            pending_norm[0]()
                            pending_norm[0] = None
                        if step == LAG:
                            # AV accumulators allocated AFTER the deferred
                            # extract so the ps_o ring never hands the psr
                            # tiles a slot aliasing a live accumulator
                            psoA = ps_op.tile([EH, SW], F32, tag="ps_o",
                                              name="psoA")
                            psoB = ps_op.tile([EH, SW], F32, tag="ps_o",
                                              name="psoB")
                        if step < nk:
                            kt = step
                            # columns < q0 of a diagonal tile are fully
                            # masked: skip them entirely; the [128,128]
                            # block at the diagonal is masked on DVE after
                            # the exp.
                            q0 = max(0, kt * P - s * SW)
                            diag = kt >= (SW // P) * s
                            ps = ps_sp.tile([P, 2, SW], F32, tag="ps_s",
                                            name="ps")
                            if s == 0:
                                u = up.tile([P, 2, SW], BF16, tag="u",
                                            name="u")
                                u_ring[kt] = u
                            elif kt % 2 == 0:
                                # fp8 u for a k-chunk PAIR: [p, head, kt2, q]
                                u8 = up.tile([P, 2, 2, SW], F8, tag="u",
                                             name="u8")
                                u_ring[kt // 2] = u8
                            else:
                                u8 = u_ring[kt // 2]
                                if diag:
                                    # odd diag chunk: cols [q0_even, q0) are
                                    # fully masked but inside the pair's AV
                                    # span; zero them (exp never writes them)
                                    q0e = max(0, (kt - 1) * P - s * SW)
                                    if q0 > q0e:
                                        nc.gpsimd.memset(
                                            u8[:, :, 1, q0e:q0], 0.0)
                            if s == 0:
                                for hh in range(2):
                                    hp = hh * HD
                                    nc.tensor.matmul(
                                        ps[:, hh, q0:SW],
                                        qk0[hp:hp + HD, kct,
                                            kt * P:(kt + 1) * P],
                                        qk0[hp:hp + HD, qct, q0:SW],
                                        start=True, stop=True,
                                    )
                            else:
                                # fp8 DoubleRow scores: head quadrant at
                                # partitions 32*qd, ch split 2x32 as k-tiles
                                tx = pr // 2
                                for hh in range(2):
                                    qd = (2 * pr) % 4 + hh
                                    b0 = 32 * qd
                                    nc.tensor.matmul(
                                        ps[:, hh, q0:SW],
                                        k8[tx][b0:b0 + 32, :,
                                               kt * P:(kt + 1) * P],
                                        q8[tx][b0:b0 + 32, :,
                                               s * SW + q0:(s + 1) * SW],
                                        start=True, stop=True,
                                        perf_mode=DR,
                                        # explicit: base_partition() rejects
                                        # 96 but the ISA allows it
                                        tile_position=(b0, 0),
                                    )
                            if s == 0:
                                nc.scalar.activation(
                                    u[:, :, q0:SW], ps[:, :, q0:SW],
                                    EXPF, scale=scale,
                                )
                                if diag:
                                    nc.vector.tensor_tensor(
                                        u[:, :, q0:q0 + P],
                                        u[:, :, q0:q0 + P],
                                        tril[:], MUL,
                                    )
                            else:
                                j = kt % 2
                                nc.scalar.activation(
                                    u8[:, :, j, q0:SW], ps[:, :, q0:SW],
                                    EXPF, scale=scale, bias=nbias[:],
                                )
                                if diag:
                                    nc.vector.tensor_tensor(
                                        u8[:, :, j, q0:q0 + P],
                                        u8[:, :, j, q0:q0 + P],
                                        tril8[:], MUL,
                                    )
                        if s == 0:
                            if step >= LAG:
                                kt = step - LAG
                                u = u_ring.pop(kt)
                                q0 = max(0, kt * P - s * SW)
                                last = kt == nk - 1
                                nc.tensor.matmul(
                                    psoA[0:EH, q0:SW],
                                    v0[:, kt,
                                       (2 * pr) * EH:(2 * pr + 1) * EH],
                                    u[:, 0, q0:SW],
                                    start=(kt == 0), stop=last,
                                )
                                nc.tensor.matmul(
                                    psoB[0:EH, q0:SW],
                                    v0[:, kt,
                                       (2 * pr + 1) * EH:(2 * pr + 2) * EH],
                                    u[:, 1, q0:SW],
                                    start=(kt == 0), stop=last,
                                )
                        elif step >= LAG and (step - LAG) % 2 == 1:
                            # fp8 DoubleRow AV over the chunk pair
                            # (kt0, kt0+1): contraction 256 at 0.5 cyc/col
                            jp = (step - LAG) // 2
                            u8c = u_ring.pop(jp)
                            kt0 = 2 * jp
                            q0p = max(0, kt0 * P - s * SW)
                            for hh, pso in ((0, psoA), (1, psoB)):
                                hcol = (2 * pr + hh) * EV
                                # single full-span inst per pair: psum
                                # start/stop marking is per-2KB bank, so
                                # region-split start=True insts would wipe
                                # each other's accumulation
                                nc.tensor.matmul(
                                    pso[0:EH, q0p:SW],
                                    v8[:, kt0:kt0 + 2,
                                       hcol:hcol + EH],
                                    u8c[:, hh, :, q0p:SW],
                                    start=(kt0 == 0),
                                    stop=(kt0 == nk - 2),
                                    perf_mode=DR,
                                )
                        sites -= 1
                        pacer.auto(sites)

                    # ---- per-pair normalize ----
                    cols = slice(s * SW, (s + 1) * SW)
                    if s == LASTS and pr == 3:
                        # fully exposed tail pair: only the psum copies are
                        # emitted here; the rest of the chain is interleaved
                        # with the final projection stages below so its PE
                        # ops never block the independent ct0 stage.
                        ounAb = normp.tile([EH, SW], BF16, tag="oun",
                                           name="ounAb")
                        ounBb = normp.tile([EH, SW], BF16, tag="oun",
                                           name="ounBb")
                        nc.scalar.copy(ounAb[:], psoA[:])
                        nc.vector.tensor_copy(ounBb[:], psoB[:])

                        def tail_extract():
                            # sums to partition 0 with tiny matmuls instead
                            # of a DMA hop (saves ~2.5us of chain latency)
                            psrA = ps_op.tile([EH, SW], F32, tag="ps_o",
                                              name="psrA")
                            psrB = ps_op.tile([EH, SW], F32, tag="ps_o",
                                              name="psrB")
                            nc.tensor.matmul(psrA[0:1, :], e64[:, 0:1],
                                             ounAb[:], start=True, stop=True)
                            nc.tensor.matmul(psrB[0:1, :], e64[:, 1:2],
                                             ounBb[:], start=True, stop=True)
                            rcA = normp.tile([1, SW], BF16, tag="rc",
                                             name="rcA")
                            rcB = normp.tile([1, SW], BF16, tag="rc",
                                             name="rcB")
                            with nc.allow_low_precision(
                                    reason="softmax sums are O(100); bf16 "
                                           "recip adds ~0.2% scale error"):
                                nc.vector.reciprocal(rcA[:], psrA[0:1, :])
                                nc.vector.reciprocal(rcB[:], psrB[0:1, :])
                            bcA = normp.tile([HD, SW], BF16, tag="bc",
                                             name="bcA")
                            bcB = normp.tile([HD, SW], BF16, tag="bc",
                                             name="bcB")
                            nc.gpsimd.partition_broadcast(bcA[:], rcA[:])
                            nc.gpsimd.partition_broadcast(bcB[:], rcB[:])
                            nc.vector.tensor_tensor(
                                attnT[0:HD, pr, cols], ounAb[0:HD, :],
                                bcA[:], MUL)
                            nc.vector.tensor_tensor(
                                attnT[HD:P, pr, cols], ounBb[0:HD, :],
                                bcB[:], MUL)

                        tail_norm[0] = tail_extract
                        continue
                    # steady state: copy psum out NOW (frees the AV banks
                    # for the next pair); the rest of the chain — e64
                    # matmul sum-extract (no DMA-hop latency), reciprocal,
                    # row broadcast, multiply — is DEFERRED past the next
                    # pair's first scores so the in-order PE never stalls
                    # on the DVE copies. bf16 operands give DVE 2x modes.
                    ounA = normp.tile([EH, SW], BF16, tag="oun", name="ounA")
                    ounB = normp.tile([EH, SW], BF16, tag="oun", name="ounB")
                    nc.vector.tensor_copy(ounA[:], psoA[:])
                    nc.vector.tensor_copy(ounB[:], psoB[:])

                    def steady_extract(pr=pr, cols=cols, ounA=ounA,
                                       ounB=ounB):
                        psrA = ps_op.tile([EH, SW], F32, tag="ps_o",
                                          name="psrA")
                        psrB = ps_op.tile([EH, SW], F32, tag="ps_o",
                                          name="psrB")
                        nc.tensor.matmul(psrA[0:1, :], e64[:, 0:1],
                                         ounA[:], start=True, stop=True)
                        nc.tensor.matmul(psrB[0:1, :], e64[:, 1:2],
                                         ounB[:], start=True, stop=True)
                        rcA = normp.tile([1, SW], BF16, tag="rc",
                                         name="rcA")
                        rcB = normp.tile([1, SW], BF16, tag="rc",
                                         name="rcB")
                        with nc.allow_low_precision(
                                reason="softmax sums are O(100); bf16 "
                                       "recip adds ~0.2% scale error"):
                            nc.vector.reciprocal(rcA[:], psrA[0:1, :])
                            nc.vector.reciprocal(rcB[:], psrB[0:1, :])
                        bcA = normp.tile([HD, SW], BF16, tag="bc",
                                         name="bcA")
                        bcB = normp.tile([HD, SW], BF16, tag="bc",
                                         name="bcB")
                        nc.gpsimd.partition_broadcast(bcA[:], rcA[:])
                        nc.gpsimd.partition_broadcast(bcB[:], rcB[:])
                        nc.vector.tensor_tensor(
                            attnT[0:HD, pr, cols], ounA[0:HD, :],
                            bcA[:], MUL)
                        nc.vector.tensor_tensor(
                            attnT[HD:P, pr, cols], ounB[0:HD, :],
                            bcB[:], MUL)

                    pending_norm[0] = steady_extract

                if s != LASTS:
                    pacer.drain()
                else:
                    tail_pacer = pacer

            # ------------- tail: strip 3 projection -----------------------
            # Stay inside the shared pools (opening a new psum pool is a
            # full drain barrier): six concurrent chains — two [P,512] in
            # mm, plus both co-halves packed into each [P,2,512] score
            # tile — run stage-major so everything except the ct=3 stage
            # overlaps the tail pair's normalize chain; the last two
            # chains follow.
            tts = list(range(4 * LASTS, 4 * LASTS + 4))
            chains = []   # (tt, co, psum_ap)
            for i in range(2):
                ps6 = ps_sp.tile([P, 2, SW], F32, tag="ps_s",
                                 name=f"psf6_{i}")
                chains.append((tts[i], 0, ps6[:, 0, :]))
                chains.append((tts[i], 1, ps6[:, 1, :]))

            def proj_stage(chain_list, ct):
                for tt, co, psy in chain_list:
                    nc.tensor.matmul(
                        psy,
                        attnT[:, ct, tt * P:(tt + 1) * P],
                        wp_sb[:, ct, co * 512:(co + 1) * 512],
                        start=(ct == 0), stop=(ct == 3),
                    )

            proj_stage(chains, 0)
            tail_norm[0]()   # extract/recip/broadcast/mult, off-PE mostly
            proj_stage(chains, 1)
            proj_stage(chains, 2)
            # held-back fillers bridge the normalize chain; they rotate the
            # mm slots, so the mm-hosted tail chains allocate only after.
            tail_pacer.drain()
            # tt15 in the AV banks (free after the sum-extract recips),
            # tt14 in mm; their ct0-2 stages also cover the chain latency.
            chains_o = []
            for i in range(2):
                pso6 = ps_op.tile([P, SW], F32, tag="ps_o", name=f"psfo_{i}")
                chains_o.append((tts[3], i, pso6[:]))
            chains_mm = []
            for i in range(2):
                psm = mmp.tile([P, 512], F32, tag="mm", name=f"psf2_{i}")
                chains_mm.append((tts[2], i, psm[:]))
            for ct in range(3):
                proj_stage(chains_o, ct)
                proj_stage(chains_mm, ct)
            yts = {}

            def proj_drain(tt, co, psy, i):
                if tt not in yts:
                    yts[tt] = ystagep.tile([P, C], BF16, tag="yt",
                                           name=f"ytf_{tt}")
                yt = yts[tt]
                if i % 2 == 0:
                    nc.scalar.copy(yt[:, co * 512:(co + 1) * 512], psy)
                else:
                    nc.vector.tensor_copy(
                        yt[:, co * 512:(co + 1) * 512], psy)
                if co == 1:
                    nc.sync.dma_start(y_r[:, tt, :], yt[:])

            proj_stage(chains, 3)
            # tts 12/13 live in single [P,2,512] tiles: one wide copy each
            # (ACT and DVE in parallel), DMA as soon as each lands
            for i in range(2):
                yt = ystagep.tile([P, C], BF16, tag="yt",
                                  name=f"ytf_{tts[i]}")
                ytv = yt[:].rearrange("p (a c) -> p a c", a=2)
                src = chains[2 * i][2].tensor.ap()
                if i == 0:
                    nc.scalar.copy(ytv, src)
                else:
                    nc.vector.tensor_copy(ytv, src)
                nc.sync.dma_start(y_r[:, tts[i], :], yt[:])
            proj_stage(chains_o, 3)
            proj_stage(chains_mm, 3)
            for i, (tt, co, psy) in enumerate(chains_o + chains_mm):
                proj_drain(tt, co, psy, i)
    nc.compile()
    return nc


def _host_consts():
    import ml_dtypes
    i_idx = np.arange(P, dtype=np.float32)[:, None]
    j_idx = np.arange(P, dtype=np.float32)[None, :]
    trf = (j_idx - i_idx >= 0).astype(np.float32)         # [k, q]: keep k<=q
    tr = trf.astype(ml_dtypes.bfloat16)
    tril = np.concatenate([tr, tr], axis=1)               # [P, 2*P]
    tr8 = trf.astype(ml_dtypes.float8_e4m3)
    tril8 = np.concatenate([tr8, tr8], axis=1)
    e64 = np.zeros((EH, 2), dtype=ml_dtypes.bfloat16)
    e64[HD, :] = 1
    return tril, tril8, e64


def make_in_maps(x, w_attn, b_attn, w_proj):
    import ml_dtypes
    bf = ml_dtypes.bfloat16
    f8 = ml_dtypes.float8_e4m3
    tril, tril8, e64 = _host_consts()
    # fp8 quadrant column permutation: production group gi = qk*4+tx*2+half,
    # col j -> local head tx*4 + j//32, channel (j%32) + 32*half
    j = np.arange(P)
    gidx = np.empty((8, P), dtype=np.int64)
    for gi in range(8):
        qk, tx, half = gi // 4, (gi // 2) % 2, gi % 2
        lh = tx * 4 + j // 32
        ch = (j % 32) + 32 * half
        gidx[gi] = qk * C + lh * HD + ch
    in_maps = []
    for c in range(8):
        b = c // 2
        h0 = (c % 2) * HPC
        qcols = slice(h0 * HD, h0 * HD + CL)
        kcols = slice(C + h0 * HD, C + h0 * HD + CL)
        vcols = slice(2 * C + h0 * HD, 2 * C + h0 * HD + CL)
        xt = np.ascontiguousarray(x[b].T)
        wqk = np.concatenate(
            [w_attn[:, qcols], w_attn[:, kcols]], axis=1).astype(bf)
        wv = w_attn[:, vcols].astype(bf)
        bun = np.concatenate([wv, xt[:, 0:SW].astype(bf), wqk], axis=1)
        bqk = np.concatenate([b_attn[qcols], b_attn[kcols]]).reshape(8, P).T
        cidx = (gidx + h0 * HD).reshape(-1)       # [8*128] global w cols
        wqk8 = w_attn[:, cidx].astype(f8)
        bqk8 = b_attn[cidx].reshape(8, P).T.astype(np.float32)
        in_maps.append({
            "bun": np.ascontiguousarray(bun),
            "x8": xt.astype(f8),
            "wqk8": np.ascontiguousarray(wqk8),
            "wv8": np.ascontiguousarray(w_attn[:, vcols]).astype(f8),
            "bqk8": np.ascontiguousarray(bqk8),
            "wp": np.ascontiguousarray(
                w_proj[h0 * HD:h0 * HD + CL, :]).astype(bf),
            "bqk": np.ascontiguousarray(bqk),
            "tril": tril,
            "tril8": tril8,
            "e64": e64,
        })
    return in_maps


def _get_runner():
    """Build the SPMD executor once: a cached jax.jit over 8 cores.

    Mirrors bass2jax.run_bass_via_pjrt but hoists the jit so repeated
    kernel() calls reuse the compiled executable.
    """
    if "runner" in _CACHED:
        return _CACHED["runner"]
    import jax
    import jax.numpy as jnp
    from jax.sharding import Mesh, PartitionSpec
    from jax.experimental.shard_map import shard_map
    from concourse import bass2jax
    import concourse.mybir as mybir_

    nc = _CACHED.get("nc")
    if nc is None:
        nc = _CACHED["nc"] = build_nc()
    bass2jax.install_neuronx_cc_hook()

    partition_name = (nc.partition_id_tensor.name
                      if nc.partition_id_tensor else None)
    in_names, out_names, out_avals, zero_shapes = [], [], [], []
    for alloc in nc.m.functions[0].allocations:
        if not isinstance(alloc, mybir_.MemoryLocationSet):
            continue
        name = alloc.memorylocations[0].name
        if alloc.kind == "ExternalInput":
            if name != partition_name:
                in_names.append(name)
        elif alloc.kind == "ExternalOutput":
            shape = tuple(alloc.tensor_shape)
            dtype = mybir_.dt.np(alloc.dtype)
            out_names.append(name)
            out_avals.append(jax.core.ShapedArray(shape, dtype))
            zero_shapes.append((shape, dtype))
    n_params = len(in_names)
    n_outs = len(out_names)
    all_names = in_names + out_names
    if partition_name is not None:
        all_names = all_names + [partition_name]

    def _body(*args):
        operands = list(args)
        if partition_name is not None:
            operands.append(bass2jax.partition_id_tensor())
        outs = bass2jax._bass_exec_p.bind(
            *operands,
            out_avals=tuple(out_avals),
            in_names=tuple(all_names),
            out_names=tuple(out_names),
            lowering_input_output_aliases=(),
            sim_require_finite=True,
            sim_require_nnan=True,
            nc=nc,
        )
        return tuple(outs)

    devices = jax.devices()[:8]
    mesh = Mesh(np.asarray(devices), ("core",))
    in_specs = (PartitionSpec("core"),) * (n_params + n_outs)
    out_specs = (PartitionSpec("core"),) * n_outs
    donate = tuple(range(n_params, n_params + n_outs))
    sharded = jax.jit(
        shard_map(_body, mesh=mesh, in_specs=in_specs, out_specs=out_specs,
                  check_rep=False),
        donate_argnums=donate, keep_unused=True,
    )

    def run(in_maps):
        concat_in = [
            np.concatenate([np.asarray(in_maps[c][nm]) for c in range(8)],
                           axis=0)
            for nm in in_names
        ]
        concat_zeros = [
            np.zeros((8 * s[0], *s[1:]), dt) for (s, dt) in zero_shapes
        ]
        out_arrs = sharded(*concat_in, *concat_zeros)
        return [
            {nm: np.asarray(out_arrs[i]).reshape(8, *out_avals[i].shape)[c]
             for i, nm in enumerate(out_names)}
            for c in range(8)
        ]

    _CACHED["runner"] = run
    return run


def kernel(x, w_attn, b_attn, w_proj, b_proj):
    x = np.asarray(x, dtype=np.float32)
    w_attn = np.asarray(w_attn, dtype=np.float32)
    b_attn = np.asarray(b_attn, dtype=np.float32)
    w_proj = np.asarray(w_proj, dtype=np.float32)
    b_proj = np.asarray(b_proj, dtype=np.float32)

    in_maps = make_in_maps(x, w_attn, b_attn, w_proj)
    results = None
    try:
        run = _get_runner()
        # The first (cold) execution occasionally races on input
        # streaming and corrupts one core's output (sometimes NaN,
        # sometimes silently). Clean executions are bit-deterministic,
        # so run twice and accept only a matching pair; tie-break with
        # extra runs. Device time per run is ~180us, so this is cheap.
        def _ys(r):
            return np.stack([c["y"].astype(np.float32) for c in r])

        prev = None
        for _ in range(4):
            cur = run(in_maps)
            ycur = _ys(cur)
            if not np.isfinite(ycur).all():
                continue
            if prev is not None and np.array_equal(prev[1], ycur):
                results = cur
                break
            prev = (cur, ycur)
        if results is None and prev is not None:
            results = prev[0]
    except Exception:
        results = None
    if results is None:
        # fallback: the stock SPMD runner (slower per call, same result)
        if "nc" not in _CACHED:
            _CACHED["nc"] = build_nc()
        res = run_bass_kernel_spmd(
            _CACHED["nc"], in_maps, core_ids=list(range(8)))
        results = res.results

    # v-bias contribution: probs rows sum to 1, so attn += 1 * b_v^T, and
    # (1 b_v^T) @ w_proj = row vector b_v @ w_proj added to every position.
    extra = b_attn[2 * C:] @ w_proj + b_proj  # [C]
    out = np.empty((B, T, C), dtype=np.float32)
    for b in range(B):
        out[b] = (results[2 * b]["y"].astype(np.float32)
                  + results[2 * b + 1]["y"].astype(np.float32) + extra)
    return out



# revision 128
# speedup vs baseline: 1.0007x; 1.0005x over previous
"""Causal multi-head attention (B=4, T=2048, C=1024, H=16) on 8 Trainium2 cores.

Sharding: core c handles batch b = c//2 and heads h0..h0+7 with h0 = (c%2)*8.
Each core computes QKV projection for its head slice, causal attention for its
8 heads, and a partial output projection. Host sums the two partials per batch
and adds the bias terms.

fp8e4 DoubleRow pipeline (180us HW; bf16 baseline was 237us at 94% PE):
  - fp8e4 matmuls with MatmulPerfMode.DoubleRow process 2 contraction
    k-tiles per instruction at 0.5 cycles/column -- 4x bf16 FLOP rate
    when the contraction fills 256, 2x when it is only 64 (scores).
    QKV production, scores, and AV for strips 1-3 all run this way; the
    PE drops from 226us busy to ~124us and the Activation engine's exp
    stream (~150us busy, element count fixed by the causal mask and
    psum-bank-limited instruction sizes) becomes the binding engine.
    Layouts: q8/k8 hold each head's 64 channels as 2x32 "k-tiles" on one
    partition quadrant (32-row matmuls at tile_position 0/32/64/96); v8
    packs [t-chunk, head*68] so AV pairs adjacent k-chunks as k-tiles,
    with the softmax ones-column riding at slot 64 (M=65 out rows).
  - Accuracy: rows with few attended keys cannot average away fp8 noise,
    so strip 0 (q in [0,512), n_eff as low as 1) keeps the full bf16
    path: bf16 q/k c-tiles, bf16 v chunks 0-3, bf16 u. Strips 1-3
    (n_eff >= ~190) run raw fp8: max-abs rel err 9.3e-3 vs the 2e-2
    gate (bf16 baseline was 4.1e-3). exp writes u as fp8 with bias -2
    so e^s stays below fp8e4's 240 max; normalization cancels the bias.
  - Strip order (1, 2, 0, 3): the fp8 stream (wqk8, x8, wv8) is DMA'd
    first and strip-1 scores start ~10us in, keeping ACT saturated from
    then on; the bf16 bundle for strip 0 streams in the fp8 strips'
    shadow and its production runs as strip-1/2 filler matmuls. Strip 3
    stays last: its long exp tail hides all deferred projection work.
  - Per-pair softmax normalize: psum -> bf16 oun copies (frees the AV
    banks), then a DEFERRED chain -- e64-matmul sum extract (no DMA
    hop), bf16 reciprocal + row broadcast + multiply (DVE 2x modes) --
    emitted after the NEXT pair's first scores so the in-order PE never
    stalls behind DVE queueing at pair boundaries.
  - One psum pool set for the whole program (pool open/close is a full
    drain barrier): 2x scores [P,2,512] (4 banks) + 2x AV [65,512]
    (2 banks, rotating with the deferred sum-extract tiles) + 2 filler
    banks. The projection tail stage-majors across the same banks with
    held-back fillers bridging the tail pair's normalize latency.
  - y is stored bf16 (halves output DMA); host sums partials in f32 and
    folds the v-bias via b_v @ w_proj (softmax rows sum to 1).
"""

import os
import sys
import numpy as np

sys.path.insert(0, "/opt/trn_rl_repo")

import concourse.bass as bass  # noqa: E402
import concourse.bacc as bacc  # noqa: E402
import concourse.mybir as mybir  # noqa: E402
from concourse.bass_utils import run_bass_kernel_spmd  # noqa: E402
from concourse.tile import TileContext  # noqa: E402

B, T, C, H = 4, 2048, 1024, 16
HD = C // H          # 64 head dim
HPC = 8              # heads per core
P = 128
NT = T // P          # 16 t-chunks of 128
SW = 512             # strip width (q and t strips)
NS = T // SW         # 4 strips
KC = C // P          # 8 contraction chunks for QKV
CL = HPC * HD        # 512 local channels per section
EH = HD + 1          # 65: head slot width in v (value cols + ones col)
F32 = mybir.dt.float32
BF16 = mybir.dt.bfloat16
F8 = mybir.dt.float8e4
DR = mybir.MatmulPerfMode.DoubleRow
EV = 68          # v8 head slot: 64 v cols + ones col at 64 + 3 pad (4B align)
EXPF = mybir.ActivationFunctionType.Exp
MUL = mybir.AluOpType.mult
DIV = mybir.AluOpType.divide
EBIAS = -4.0     # exp(s*scale + EBIAS) must stay < 240 (fp8e4 max):
                 # observed max s*scale is 8.06 on the axon-backend input
                 # draw (6.2 on the CPU draw); -4 leaves >1 margin incl.
                 # fp8 q/k quantization jitter. Normalization cancels it.

_CACHED = {}


def build_nc():
    nc = bacc.Bacc("TRN2", target_bir_lowering=False, debug=False)

    # bundle row c = [x_strip0[c, 0:512] | wv[c, :] | wqk[c, :]] so each kc
    # contraction chunk arrives in ONE 4KB-per-partition DMA (HWDGE is a
    # serial 625ns-per-DMA resource; descriptor splits are what cost time).
    bun_d = nc.dram_tensor("bun", [C, 4 * CL], BF16, kind="ExternalInput")
    # fp8 inputs for the DoubleRow QKV production (strips 1-3)
    x8_d = nc.dram_tensor("x8", [C, T], F8, kind="ExternalInput")
    wqk8_d = nc.dram_tensor("wqk8", [C, 8 * P], F8, kind="ExternalInput")
    wv8_d = nc.dram_tensor("wv8", [C, CL], F8, kind="ExternalInput")
    bqk8_d = nc.dram_tensor("bqk8", [P, 8], F32, kind="ExternalInput")
    wp_d = nc.dram_tensor("wp", [CL, C], BF16, kind="ExternalInput")
    bqk_d = nc.dram_tensor("bqk", [P, 8], F32, kind="ExternalInput")
    tril_d = nc.dram_tensor("tril", [P, 2 * P], BF16, kind="ExternalInput")
    tril8_d = nc.dram_tensor("tril8", [P, 2 * P], F8, kind="ExternalInput")
    e64_d = nc.dram_tensor("e64", [EH, 2], BF16, kind="ExternalInput")
    y_d = nc.dram_tensor("y", [T, C], BF16, kind="ExternalOutput")

    bun_r = bun_d.ap().rearrange("(kc p) c -> p kc c", p=P)     # [128, 8, 2048]
    x8_r = x8_d.ap().rearrange("(kc p) t -> p kc t", p=P)       # [128, 8, 2048]
    wqk8_r = wqk8_d.ap().rearrange("(kc p) c -> p kc c", p=P)   # [128, 8, 1024]
    wv8_r = wv8_d.ap().rearrange("(kc p) c -> p kc c", p=P)     # [128, 8, 512]
    wp_r = wp_d.ap().rearrange("(ct p) c -> p ct c", p=P)       # [128, 4, 1024]
    y_r = y_d.ap().rearrange("(tt p) c -> p tt c", p=P)         # [128, 16, 1024]

    scale = float(HD) ** -0.5
    LAG = 2

    with TileContext(nc) as tc, \
         tc.tile_pool(name="const", bufs=1) as constp, \
         tc.tile_pool(name="big", bufs=1) as bigp, \
         tc.tile_pool(name="u_pool", bufs=int(os.environ.get("K_UBUFS", "12"))) as up, \
         tc.tile_pool(name="norm", bufs=int(os.environ.get("K_NBUFS", "6"))) as normp, \
         tc.tile_pool(name="ystage", bufs=int(os.environ.get("K_YBUFS", "4"))) as ystagep:

        # strip-0 q,k in bf16 c-tile layout (accurate scores for small-n_eff
        # rows); strips 1-3 q,k live in fp8 quadrant layout (q8/k8)
        qk0 = bigp.tile([P, 8, SW], BF16)     # c-tiles 0-3 = qT, 4-7 = kT
        # fp8 quadrant layout: tile X holds heads 4X..4X+3; head h on
        # partitions 32*(h%4)..+32, dim1 = ch-half (2x32), dim2 = t.
        # Feeds 32-partition DoubleRow scores matmuls.
        q8 = [bigp.tile([P, 2, T], F8, name=f"q8_{x}") for x in range(2)]
        k8 = [bigp.tile([P, 2, T], F8, name=f"k8_{x}") for x in range(2)]
        # strip-0 AV runs bf16 (small n_eff rows need accuracy): v chunks 0-3
        v0 = bigp.tile([P, 4, HPC * EH], BF16)
        v0_heads = v0[:].rearrange("p t (h e) -> p t h e", e=EH)
        # strips 1-3 AV runs fp8 DoubleRow: all 16 chunks, ones col at 64
        v8 = bigp.tile([P, NT, HPC * EV], F8)
        v8_heads = v8[:].rearrange("p t (h e) -> p t h e", e=EV)
        attnT = bigp.tile([P, 4, T], BF16)
        bun_sb = bigp.tile([P, KC, 4 * CL], BF16)  # [x0 | wv | wqk] per kc
        x8_sb = bigp.tile([P, KC, T], F8)
        wqk8_sb = bigp.tile([P, KC, 8 * P], F8)
        wv8_sb = bigp.tile([P, KC, CL], F8)
        wp_sb = bigp.tile([P, 4, C], BF16)
        bqk = constp.tile([P, 8], F32)
        bqk8 = constp.tile([P, 8], F32)
        tril = constp.tile([P, 2, P], BF16)
        tril8 = constp.tile([P, 2, P], F8)
        e64 = constp.tile([EH, 2], BF16)
        nbias = constp.tile([P, 1], F32)
        wv_sb = bun_sb[:, :, 0:CL]
        xts0 = bun_sb[:, :, CL:CL + SW]
        wqk_sb = bun_sb[:, :, 2 * CL:4 * CL]

        # -------------------------- input DMAs ---------------------------
        # Strips run in order (1, 2, 3, 0): the fp8 stream goes FIRST so
        # the ACT-saturating fp8 strips start exp'ing ~10us in, and the
        # bf16 strip-0 bundle (consumed last) streams in their shadow.
        # Tiny consts ride the idle Pool SWDGE queue; the wtiny memset
        # leads so the p-state warm-up chain can start immediately.
        wtiny = constp.tile([2, SW], BF16)
        nc.gpsimd.memset(wtiny[:], 0.0)
        nc.gpsimd.dma_start(bqk[:], bqk_d[:])
        nc.gpsimd.dma_start(bqk8[:], bqk8_d[:])
        nc.gpsimd.dma_start(
            tril[:], tril_d.ap().rearrange("p (h q) -> p h q", h=2))
        nc.gpsimd.dma_start(
            tril8[:], tril8_d.ap().rearrange("p (h q) -> p h q", h=2))
        nc.gpsimd.dma_start(e64[:], e64_d[:])
        nc.gpsimd.memset(nbias[:], EBIAS)
        nc.gpsimd.memset(v0_heads[:, :, :, HD], 1.0)
        nc.gpsimd.memset(v8_heads[:, :, :, HD], 1.0)
        # fp8 stream, ordered by first consumption: wqk8 k-groups (early
        # k0 production), x8 span 0, wqk8 q-groups, x8 span 1, wv8
        # (v8 chunks), x8 spans 2/3
        nc.sync.dma_start(wqk8_sb[:, 0:4, 4 * P:], wqk8_r[:, 0:4, 4 * P:])
        nc.sync.dma_start(wqk8_sb[:, 4:8, 4 * P:], wqk8_r[:, 4:8, 4 * P:])
        nc.sync.dma_start(x8_sb[:, 0:4, 0:SW], x8_r[:, 0:4, 0:SW])
        nc.sync.dma_start(x8_sb[:, 4:8, 0:SW], x8_r[:, 4:8, 0:SW])
        nc.sync.dma_start(wqk8_sb[:, :, 0:4 * P], wqk8_r[:, :, 0:4 * P])
        nc.sync.dma_start(x8_sb[:, 0:4, SW:2 * SW], x8_r[:, 0:4, SW:2 * SW])
        nc.sync.dma_start(x8_sb[:, 4:8, SW:2 * SW], x8_r[:, 4:8, SW:2 * SW])
        nc.sync.dma_start(wv8_sb[:], wv8_r)
        for sp in (2, 3):
            nc.sync.dma_start(x8_sb[:, :, sp * SW:(sp + 1) * SW],
                              x8_r[:, :, sp * SW:(sp + 1) * SW])
        # bf16 bundle for strip-0 production (consumed by strip-2 fillers)
        for kc in range(KC):
            nc.sync.dma_start(bun_sb[:, kc, :], bun_r[:, kc, :])
        nc.sync.dma_start(wp_sb[:], wp_r)

        # warm the exp table during the DMA lead-in (LoadActFuncSet is lazy
        # and otherwise lands on the first-scores critical path)
        warm = constp.tile([1, 2], F32)
        nc.gpsimd.memset(warm[:], 0.0)
        nc.scalar.activation(warm[:], warm[:], EXPF)

        with tc.tile_pool(name="mm", bufs=2, space="PSUM") as mmp, \
             tc.tile_pool(name="ps_s", bufs=2, space="PSUM") as ps_sp, \
             tc.tile_pool(name="ps_o", bufs=2, space="PSUM") as ps_op:

            # ---------------- op generators -------------------------------
            def gen_qk8(s, which, txs=(0, 1), c0=0, c1=SW):
                """fp8 DoubleRow q/k production for strip-span s into the
                quadrant layout (4x contraction rate vs bf16); one yield
                per instruction. c0/c1 select a column sub-span (used to
                bootstrap the first k chunks during the DMA lead-in)."""
                cols = slice(s * SW + c0, s * SW + c1)
                for qk in which:           # 0 = q, 1 = k
                    dst = q8 if qk == 0 else k8
                    for tx in txs:
                        for half in range(2):
                            gi = qk * 4 + tx * 2 + half
                            psq = mmp.tile([P, c1 - c0], F32, tag="mm",
                                           name="psq8")
                            for kcp in range(KC // 2):
                                nc.tensor.matmul(
                                    psq[:],
                                    wqk8_sb[:, 2 * kcp:2 * kcp + 2,
                                            gi * P:(gi + 1) * P],
                                    x8_sb[:, 2 * kcp:2 * kcp + 2, cols],
                                    start=(kcp == 0),
                                    stop=(kcp == KC // 2 - 1),
                                    perf_mode=DR,
                                )
                                yield
                            nc.vector.tensor_scalar_add(
                                dst[tx][:, half, cols], psq[:],
                                bqk8[:, gi:gi + 1])
                            yield

            def gen_v8(tchunks):
                """fp8 DoubleRow v production."""
                for tch in tchunks:
                    psv = mmp.tile([P, CL], F32, tag="mm", name="psv8")
                    for kcp in range(KC // 2):
                        nc.tensor.matmul(
                            psv[:],
                            x8_sb[:, 2 * kcp:2 * kcp + 2,
                                  tch * P:(tch + 1) * P],
                            wv8_sb[:, 2 * kcp:2 * kcp + 2, :],
                            start=(kcp == 0), stop=(kcp == KC // 2 - 1),
                            perf_mode=DR,
                        )
                        yield
                    nc.vector.tensor_copy(
                        v8_heads[:, tch, :, 0:HD],
                        psv[:].rearrange("p (h d) -> p h d", d=HD),
                    )
                    yield

            def gen_qk0(cts):
                """bf16 strip-0 q/k c-tile production from the bundle;
                consumed only by strip 0, which runs last."""
                for ct in cts:
                    psq = mmp.tile([P, SW], F32, tag="mm", name="psq0r")
                    for kc in range(KC):
                        nc.tensor.matmul(
                            psq[:],
                            wqk_sb[:, kc, ct * P:(ct + 1) * P],
                            xts0[:, kc, :],
                            start=(kc == 0), stop=(kc == KC - 1),
                        )
                        yield
                    nc.vector.tensor_scalar_add(
                        qk0[:, ct, :], psq[:], bqk[:, ct:ct + 1])
                    yield

            def gen_v0():
                """bf16 v production for strip-0's chunks 0-3 (accurate
                path for the small-n_eff rows)."""
                for tt in range(4):
                    psv = mmp.tile([P, CL], F32, tag="mm", name="psv0")
                    for kc in range(KC):
                        nc.tensor.matmul(
                            psv[:],
                            xts0[:, kc, tt * P:(tt + 1) * P],
                            wv_sb[:, kc, :],
                            start=(kc == 0), stop=(kc == KC - 1),
                        )
                        yield
                    nc.vector.tensor_copy(
                        v0_heads[:, tt, :, 0:HD],
                        psv[:].rearrange("p (h d) -> p h d", d=HD),
                    )
                    yield

            def gen_proj(s):
                """Output projection for strip s; one yield per matmul.
                Both halves of a t-chunk share one [P, 1024] staging tile so
                each t-chunk costs a single (2KB/descriptor) y DMA."""
                for tt4 in range(SW // P):
                    tt = s * (SW // P) + tt4
                    yt = ystagep.tile([P, C], BF16, tag="yt")
                    for co in range(2):
                        psy = mmp.tile([P, 512], F32, tag="mm", name="psy")
                        for ct in range(4):
                            nc.tensor.matmul(
                                psy[:],
                                attnT[:, ct, tt * P:(tt + 1) * P],
                                wp_sb[:, ct, co * 512:(co + 1) * 512],
                                start=(ct == 0), stop=(ct == 3),
                            )
                            yield
                        nc.vector.tensor_copy(
                            yt[:, co * 512:(co + 1) * 512], psy[:])
                        yield
                    nc.sync.dma_start(y_r[:, tt, :], yt[:])

            class Pacer:
                def __init__(self, gens_counts, reserve=0):
                    self.gens = [g for g, n in gens_counts]
                    self.remaining = sum(n for g, n in gens_counts)
                    self.reserve = reserve

                def pump(self, n):
                    for _ in range(n):
                        while self.gens:
                            try:
                                next(self.gens[0])
                                self.remaining -= 1
                                break
                            except StopIteration:
                                self.gens.pop(0)
                        if not self.gens:
                            self.remaining = 0
                            return

                def auto(self, sites_left):
                    # spread the unreserved remainder over remaining sites
                    free = self.remaining - self.reserve
                    if free <= 0 or sites_left <= 0:
                        return
                    self.pump(-(-free // sites_left))

                def drain(self):
                    while self.gens:
                        self.pump(1)

            BPUMP = int(os.environ.get("K_BPUMP", "7"))
            tail_norm = [None]
            pending_norm = [None]
            LAGS = [int(v) for v in
                    os.environ.get("K_LAGS", "4,4,4,4").split(",")]
            RESV = int(os.environ.get("K_RESV", "20"))

            # PE p-state warm-up: dummy matmuls run contiguously INTO the
            # first production chain so the 3us ramp to 2.4GHz completes
            # and persists (idle resets it)
            NW = int(os.environ.get("K_NW", "0"))
            if NW:
                pswm = mmp.tile([2, SW], F32, tag="mm", name="pswm")
                for _ in range(NW):
                    nc.tensor.matmul(pswm[0:2, :], wtiny[:, 0:2], wtiny[:],
                                     start=True, stop=True)

            # ---- early direct production: strip-1's scores inputs ------
            # (k span 0, q span 1, k span 1) in DMA-arrival order; the PE
            # is otherwise idle while the fp8 stream lands.
            for g in (gen_qk8(0, (1,)), gen_qk8(1, (0,)),
                      gen_qk8(1, (1,))):
                for _ in g:
                    pass

            # ---------------- fused attention pipeline --------------------
            # Strip order (1, 2, 0, 3): the fp8 strips keep ACT (exp, the
            # bottleneck engine) saturated from ~10us on; bf16 strip 0,
            # whose bundle arrives last on the DMA queue, slots in third
            # (its production hidden in strip-1/2 fillers), and strip 3
            # stays last so the proj tail interleave applies unchanged.
            LASTS = 3
            for s in (1, 2, 0, 3):
                LAG = LAGS[s]
                gens = []
                if s == 1:
                    gens.append((gen_v8(range(0, 8)), 40))
                    gens.append((gen_qk8(2, (0, 1)), 40))
                    gens.append((gen_v8(range(8, 12)), 20))
                elif s == 2:
                    gens.append((gen_qk0(range(8)), 72))
                    gens.append((gen_v0(), 36))
                elif s == 0:
                    gens.append((gen_qk8(3, (0, 1)), 40))
                else:
                    gens.append((gen_v8(range(12, 16)), 20))
                    for ps_ in (1, 2, 0):
                        gens.append((gen_proj(ps_), 40))
                # the last strip holds back ~20 filler matmuls to cover the
                # tail pair's normalize latency during the final projection
                pacer = Pacer(gens, reserve=RESV if s == LASTS else 0)
                nk = (SW // P) * (s + 1)
                sites = 4 * (nk + LAG)

                for pr in range(4):  # head pair (2pr, 2pr+1)
                    qct, kct = pr, 4 + pr
                    # pair 0 of a strip: nothing is in flight yet — big
                    # pre-pumps would just delay the strip's first scores
                    pacer.pump(int(os.environ.get('K_P0', '2'))
                               if pr == 0 else BPUMP)
                    psoA = psoB = None
                    u_ring = {}
                    for step in range(nk + LAG):
                        if (step == int(os.environ.get('K_FLS', '3'))
                                and pending_norm[0] is not None):
                            # previous pair's normalize extract, deferred
                            # here so its psr matmuls never stall the PE
                            # (the oun copies have long since drained)
                            pending_norm[0]()
                            pending_norm[0] = None
                        if step == LAG:
                            # AV accumulators allocated AFTER the deferred
                            # extract so the ps_o ring never hands the psr
                            # tiles a slot aliasing a live accumulator
                            psoA = ps_op.tile([EH, SW], F32, tag="ps_o",
                                              name="psoA")
                            psoB = ps_op.tile([EH, SW], F32, tag="ps_o",
                                              name="psoB")
                        if step < nk:
                            kt = step
                            # columns < q0 of a diagonal tile are fully
                            # masked: skip them entirely; the [128,128]
                            # block at the diagonal is masked on DVE after
                            # the exp.
                            q0 = max(0, kt * P - s * SW)
                            diag = kt >= (SW // P) * s
                            ps = ps_sp.tile([P, 2, SW], F32, tag="ps_s",
                                            name="ps")
                            if s == 0:
                                u = up.tile([P, 2, SW], BF16, tag="u",
                                            name="u")
                                u_ring[kt] = u
                            elif kt % 2 == 0:
                                # fp8 u for a k-chunk PAIR: [p, head, kt2, q]
                                u8 = up.tile([P, 2, 2, SW], F8, tag="u",
                                             name="u8")
                                u_ring[kt // 2] = u8
                            else:
                                u8 = u_ring[kt // 2]
                                if diag:
                                    # odd diag chunk: cols [q0_even, q0) are
                                    # fully masked but inside the pair's AV
                                    # span; zero them (exp never writes them)
                                    q0e = max(0, (kt - 1) * P - s * SW)
                                    if q0 > q0e:
                                        nc.gpsimd.memset(
                                            u8[:, :, 1, q0e:q0], 0.0)
                            if s == 0:
                                for hh in range(2):
                                    hp = hh * HD
                                    nc.tensor.matmul(
                                        ps[:, hh, q0:SW],
                                        qk0[hp:hp + HD, kct,
                                            kt * P:(kt + 1) * P],
                                        qk0[hp:hp + HD, qct, q0:SW],
                                        start=True, stop=True,
                                    )
                            else:
                                # fp8 DoubleRow scores: head quadrant at
                                # partitions 32*qd, ch split 2x32 as k-tiles
                                tx = pr // 2
                                for hh in range(2):
                                    qd = (2 * pr) % 4 + hh
                                    b0 = 32 * qd
                                    nc.tensor.matmul(
                                        ps[:, hh, q0:SW],
                                        k8[tx][b0:b0 + 32, :,
                                               kt * P:(kt + 1) * P],
                                        q8[tx][b0:b0 + 32, :,
                                               s * SW + q0:(s + 1) * SW],
                                        start=True, stop=True,
                                        perf_mode=DR,
                                        # explicit: base_partition() rejects
                                        # 96 but the ISA allows it
                                        tile_position=(b0, 0),
                                    )
                            if s == 0:
                                nc.scalar.activation(
                                    u[:, :, q0:SW], ps[:, :, q0:SW],
                                    EXPF, scale=scale,
                                )
                                if diag:
                                    nc.vector.tensor_tensor(
                                        u[:, :, q0:q0 + P],
                                        u[:, :, q0:q0 + P],
                                        tril[:], MUL,
                                    )
                            else:
                                j = kt % 2
                                nc.scalar.activation(
                                    u8[:, :, j, q0:SW], ps[:, :, q0:SW],
                                    EXPF, scale=scale, bias=nbias[:],
                                )
                                if diag:
                                    nc.vector.tensor_tensor(
                                        u8[:, :, j, q0:q0 + P],
                                        u8[:, :, j, q0:q0 + P],
                                        tril8[:], MUL,
                                    )
                        if s == 0:
                            if step >= LAG:
                                kt = step - LAG
                                u = u_ring.pop(kt)
                                q0 = max(0, kt * P - s * SW)
                                last = kt == nk - 1
                                nc.tensor.matmul(
                                    psoA[0:EH, q0:SW],
                                    v0[:, kt,
                                       (2 * pr) * EH:(2 * pr + 1) * EH],
                                    u[:, 0, q0:SW],
                                    start=(kt == 0), stop=last,
                                )
                                nc.tensor.matmul(
                                    psoB[0:EH, q0:SW],
                                    v0[:, kt,
                                       (2 * pr + 1) * EH:(2 * pr + 2) * EH],
                                    u[:, 1, q0:SW],
                                    start=(kt == 0), stop=last,
                                )
                        elif step >= LAG and (step - LAG) % 2 == 1:
                            # fp8 DoubleRow AV over the chunk pair
                            # (kt0, kt0+1): contraction 256 at 0.5 cyc/col
                            jp = (step - LAG) // 2
                            u8c = u_ring.pop(jp)
                            kt0 = 2 * jp
                            q0p = max(0, kt0 * P - s * SW)
                            for hh, pso in ((0, psoA), (1, psoB)):
                                hcol = (2 * pr + hh) * EV
                                # single full-span inst per pair: psum
                                # start/stop marking is per-2KB bank, so
                                # region-split start=True insts would wipe
                                # each other's accumulation
                                nc.tensor.matmul(
                                    pso[0:EH, q0p:SW],
                                    v8[:, kt0:kt0 + 2,
                                       hcol:hcol + EH],
                                    u8c[:, hh, :, q0p:SW],
                                    start=(kt0 == 0),
                                    stop=(kt0 == nk - 2),
                                    perf_mode=DR,
                                )
                        sites -= 1
                        pacer.auto(sites)

                    # ---- per-pair normalize ----
                    cols = slice(s * SW, (s + 1) * SW)
                    if s == LASTS and pr == 3:
                        # fully exposed tail pair: only the psum copies are
                        # emitted here; the rest of the chain is interleaved
                        # with the final projection stages below so its PE
                        # ops never block the independent ct0 stage.
                        ounAb = normp.tile([EH, SW], BF16, tag="oun",
                                           name="ounAb")
                        ounBb = normp.tile([EH, SW], BF16, tag="oun",
                                           name="ounBb")
                        nc.scalar.copy(ounAb[:], psoA[:])
                        nc.vector.tensor_copy(ounBb[:], psoB[:])

                        def tail_extract():
                            # sums to partition 0 with tiny matmuls instead
                            # of a DMA hop (saves ~2.5us of chain latency)
                            psrA = ps_op.tile([EH, SW], F32, tag="ps_o",
                                              name="psrA")
                            psrB = ps_op.tile([EH, SW], F32, tag="ps_o",
                                              name="psrB")
                            nc.tensor.matmul(psrA[0:1, :], e64[:, 0:1],
                                             ounAb[:], start=True, stop=True)
                            nc.tensor.matmul(psrB[0:1, :], e64[:, 1:2],
                                             ounBb[:], start=True, stop=True)
                            rcA = normp.tile([1, SW], BF16, tag="rc",
                                             name="rcA")
                            rcB = normp.tile([1, SW], BF16, tag="rc",
                                             name="rcB")
                            with nc.allow_low_precision(
                                    reason="softmax sums are O(100); bf16 "
                                           "recip adds ~0.2% scale error"):
                                nc.vector.reciprocal(rcA[:], psrA[0:1, :])
                                nc.vector.reciprocal(rcB[:], psrB[0:1, :])
                            bcA = normp.tile([HD, SW], BF16, tag="bc",
                                             name="bcA")
                            bcB = normp.tile([HD, SW], BF16, tag="bc",
                                             name="bcB")
                            nc.gpsimd.partition_broadcast(bcA[:], rcA[:])
                            nc.gpsimd.partition_broadcast(bcB[:], rcB[:])
                            nc.vector.tensor_tensor(
                                attnT[0:HD, pr, cols], ounAb[0:HD, :],
                                bcA[:], MUL)
                            nc.vector.tensor_tensor(
                                attnT[HD:P, pr, cols], ounBb[0:HD, :],
                                bcB[:], MUL)

                        tail_norm[0] = tail_extract
                        continue
                    # steady state: copy psum out NOW (frees the AV banks
                    # for the next pair); the rest of the chain — e64
                    # matmul sum-extract (no DMA-hop latency), reciprocal,
                    # row broadcast, multiply — is DEFERRED past the next
                    # pair's first scores so the in-order PE never stalls
                    # on the DVE copies. bf16 operands give DVE 2x modes.
                    ounA = normp.tile([EH, SW], BF16, tag="oun", name="ounA")
                    ounB = normp.tile([EH, SW], BF16, tag="oun", name="ounB")
                    nc.vector.tensor_copy(ounA[:], psoA[:])
                    nc.vector.tensor_copy(ounB[:], psoB[:])

                    def steady_extract(pr=pr, cols=cols, ounA=ounA,
                                       ounB=ounB):
                        psrA = ps_op.tile([EH, SW], F32, tag="ps_o",
                                          name="psrA")
                        psrB = ps_op.tile([EH, SW], F32, tag="ps_o",
                                          name="psrB")
                        nc.tensor.matmul(psrA[0:1, :], e64[:, 0:1],
                                         ounA[:], start=True, stop=True)
                        nc.tensor.matmul(psrB[0:1, :], e64[:, 1:2],
                                         ounB[:], start=True, stop=True)
                        rcA = normp.tile([1, SW], BF16, tag="rc",
                                         name="rcA")
                        rcB = normp.tile([1, SW], BF16, tag="rc",
                                         name="rcB")
                        with nc.allow_low_precision(
                                reason="softmax sums are O(100); bf16 "
                                       "recip adds ~0.2% scale error"):
                            nc.vector.reciprocal(rcA[:], psrA[0:1, :])
                            nc.vector.reciprocal(rcB[:], psrB[0:1, :])
                        bcA = normp.tile([HD, SW], BF16, tag="bc",
                                         name="bcA")
                        bcB = normp.tile([HD, SW], BF16, tag="bc",
                                         name="bcB")
                        nc.gpsimd.partition_broadcast(bcA[:], rcA[:])
                        nc.gpsimd.partition_broadcast(bcB[:], rcB[:])
                        nc.vector.tensor_tensor(
                            attnT[0:HD, pr, cols], ounA[0:HD, :],
                            bcA[:], MUL)
                        nc.vector.tensor_tensor(
                            attnT[HD:P, pr, cols], ounB[0:HD, :],
                            bcB[:], MUL)

                    pending_norm[0] = steady_extract

                if s != LASTS:
                    pacer.drain()
                else:
                    tail_pacer = pacer

            # ------------- tail: strip 3 projection -----------------------
            # Stay inside the shared pools (opening a new psum pool is a
            # full drain barrier): six concurrent chains — two [P,512] in
            # mm, plus both co-halves packed into each [P,2,512] score
            # tile — run stage-major so everything except the ct=3 stage
            # overlaps the tail pair's normalize chain; the last two
            # chains follow.
            tts = list(range(4 * LASTS, 4 * LASTS + 4))
            chains = []   # (tt, co, psum_ap)
            for i in range(2):
                ps6 = ps_sp.tile([P, 2, SW], F32, tag="ps_s",
                                 name=f"psf6_{i}")
                chains.append((tts[i], 0, ps6[:, 0, :]))
                chains.append((tts[i], 1, ps6[:, 1, :]))

            def proj_stage(chain_list, ct):
                for tt, co, psy in chain_list:
                    nc.tensor.matmul(
                        psy,
                        attnT[:, ct, tt * P:(tt + 1) * P],
                        wp_sb[:, ct, co * 512:(co + 1) * 512],
                        start=(ct == 0), stop=(ct == 3),
                    )

            proj_stage(chains, 0)
            tail_norm[0]()   # extract/recip/broadcast/mult, off-PE mostly
            proj_stage(chains, 1)
            proj_stage(chains, 2)
            # held-back fillers bridge the normalize chain; they rotate the
            # mm slots, so the mm-hosted tail chains allocate only after.
            tail_pacer.drain()
            # tt15 in the AV banks (free after the sum-extract recips),
            # tt14 in mm; their ct0-2 stages also cover the chain latency.
            chains_o = []
            for i in range(2):
                pso6 = ps_op.tile([P, SW], F32, tag="ps_o", name=f"psfo_{i}")
                chains_o.append((tts[3], i, pso6[:]))
            chains_mm = []
            for i in range(2):
                psm = mmp.tile([P, 512], F32, tag="mm", name=f"psf2_{i}")
                chains_mm.append((tts[2], i, psm[:]))
            for ct in range(3):
                proj_stage(chains_o, ct)
                proj_stage(chains_mm, ct)
            yts = {}

            def proj_drain(tt, co, psy, i):
                if tt not in yts:
                    yts[tt] = ystagep.tile([P, C], BF16, tag="yt",
                                           name=f"ytf_{tt}")
                yt = yts[tt]
                if i % 2 == 0:
                    nc.scalar.copy(yt[:, co * 512:(co + 1) * 512], psy)
                else:
                    nc.vector.tensor_copy(
                        yt[:, co * 512:(co + 1) * 512], psy)
                if co == 1:
                    nc.sync.dma_start(y_r[:, tt, :], yt[:])

            proj_stage(chains, 3)
            # tts 12/13 live in single [P,2,512] tiles: one wide copy each
            # (ACT and DVE in parallel), DMA as soon as each lands
            for i in range(2):
                yt = ystagep.tile([P, C], BF16, tag="yt",
                                  name=f"ytf_{tts[i]}")
                ytv = yt[:].rearrange("p (a c) -> p a c", a=2)
                src = chains[2 * i][2].tensor.ap()
                if i == 0:
                    nc.scalar.copy(ytv, src)
                else:
                    nc.vector.tensor_copy(ytv, src)
                nc.sync.dma_start(y_r[:, tts[i], :], yt[:])
            proj_stage(chains_o, 3)
            proj_stage(chains_mm, 3)
            for i, (tt, co, psy) in enumerate(chains_o + chains_mm):
                proj_drain(tt, co, psy, i)
    nc.compile()
    return nc


def _host_consts():
    import ml_dtypes
    i_idx = np.arange(P, dtype=np.float32)[:, None]
    j_idx = np.arange(P, dtype=np.float32)[None, :]
    trf = (j_idx - i_idx >= 0).astype(np.float32)         # [k, q]: keep k<=q
    tr = trf.astype(ml_dtypes.bfloat16)
    tril = np.concatenate([tr, tr], axis=1)               # [P, 2*P]
    tr8 = trf.astype(ml_dtypes.float8_e4m3)
    tril8 = np.concatenate([tr8, tr8], axis=1)
    e64 = np.zeros((EH, 2), dtype=ml_dtypes.bfloat16)
    e64[HD, :] = 1
    return tril, tril8, e64


def make_in_maps(x, w_attn, b_attn, w_proj):
    import ml_dtypes
    bf = ml_dtypes.bfloat16
    f8 = ml_dtypes.float8_e4m3
    tril, tril8, e64 = _host_consts()
    # fp8 quadrant column permutation: production group gi = qk*4+tx*2+half,
    # col j -> local head tx*4 + j//32, channel (j%32) + 32*half
    j = np.arange(P)
    gidx = np.empty((8, P), dtype=np.int64)
    for gi in range(8):
        qk, tx, half = gi // 4, (gi // 2) % 2, gi % 2
        lh = tx * 4 + j // 32
        ch = (j % 32) + 32 * half
        gidx[gi] = qk * C + lh * HD + ch
    in_maps = []
    for c in range(8):
        b = c // 2
        h0 = (c % 2) * HPC
        qcols = slice(h0 * HD, h0 * HD + CL)
        kcols = slice(C + h0 * HD, C + h0 * HD + CL)
        vcols = slice(2 * C + h0 * HD, 2 * C + h0 * HD + CL)
        xt = np.ascontiguousarray(x[b].T)
        wqk = np.concatenate(
            [w_attn[:, qcols], w_attn[:, kcols]], axis=1).astype(bf)
        wv = w_attn[:, vcols].astype(bf)
        bun = np.concatenate([wv, xt[:, 0:SW].astype(bf), wqk], axis=1)
        bqk = np.concatenate([b_attn[qcols], b_attn[kcols]]).reshape(8, P).T
        cidx = (gidx + h0 * HD).reshape(-1)       # [8*128] global w cols
        wqk8 = w_attn[:, cidx].astype(f8)
        bqk8 = b_attn[cidx].reshape(8, P).T.astype(np.float32)
        in_maps.append({
            "bun": np.ascontiguousarray(bun),
            "x8": xt.astype(f8),
            "wqk8": np.ascontiguousarray(wqk8),
            "wv8": np.ascontiguousarray(w_attn[:, vcols]).astype(f8),
            "bqk8": np.ascontiguousarray(bqk8),
            "wp": np.ascontiguousarray(
                w_proj[h0 * HD:h0 * HD + CL, :]).astype(bf),
            "bqk": np.ascontiguousarray(bqk),
            "tril": tril,
            "tril8": tril8,
            "e64": e64,
        })
    return in_maps


def _get_runner():
    """Build the SPMD executor once: a cached jax.jit over 8 cores.

    Mirrors bass2jax.run_bass_via_pjrt but hoists the jit so repeated
    kernel() calls reuse the compiled executable.
    """
    if "runner" in _CACHED:
        return _CACHED["runner"]
    import jax
    import jax.numpy as jnp
    from jax.sharding import Mesh, PartitionSpec
    from jax.experimental.shard_map import shard_map
    from concourse import bass2jax
    import concourse.mybir as mybir_

    nc = _CACHED.get("nc")
    if nc is None:
        nc = _CACHED["nc"] = build_nc()
    bass2jax.install_neuronx_cc_hook()

    partition_name = (nc.partition_id_tensor.name
                      if nc.partition_id_tensor else None)
    in_names, out_names, out_avals, zero_shapes = [], [], [], []
    for alloc in nc.m.functions[0].allocations:
        if not isinstance(alloc, mybir_.MemoryLocationSet):
            continue
        name = alloc.memorylocations[0].name
        if alloc.kind == "ExternalInput":
            if name != partition_name:
                in_names.append(name)
        elif alloc.kind == "ExternalOutput":
            shape = tuple(alloc.tensor_shape)
            dtype = mybir_.dt.np(alloc.dtype)
            out_names.append(name)
            out_avals.append(jax.core.ShapedArray(shape, dtype))
            zero_shapes.append((shape, dtype))
    n_params = len(in_names)
    n_outs = len(out_names)
    all_names = in_names + out_names
    if partition_name is not None:
        all_names = all_names + [partition_name]

    def _body(*args):
        operands = list(args)
        if partition_name is not None:
            operands.append(bass2jax.partition_id_tensor())
        outs = bass2jax._bass_exec_p.bind(
            *operands,
            out_avals=tuple(out_avals),
            in_names=tuple(all_names),
            out_names=tuple(out_names),
            lowering_input_output_aliases=(),
            sim_require_finite=True,
            sim_require_nnan=True,
            nc=nc,
        )
        return tuple(outs)

    devices = jax.devices()[:8]
    mesh = Mesh(np.asarray(devices), ("core",))
    in_specs = (PartitionSpec("core"),) * (n_params + n_outs)
    out_specs = (PartitionSpec("core"),) * n_outs
    donate = tuple(range(n_params, n_params + n_outs))
    sharded = jax.jit(
        shard_map(_body, mesh=mesh, in_specs=in_specs, out_specs=out_specs,
                  check_rep=False),
        donate_argnums=donate, keep_unused=True,
    )

    def run(in_maps):
        concat_in = [
            np.concatenate([np.asarray(in_maps[c][nm]) for c in range(8)],
                           axis=0)
            for nm in in_names
        ]
        concat_zeros = [
            np.zeros((8 * s[0], *s[1:]), dt) for (s, dt) in zero_shapes
        ]
        out_arrs = sharded(*concat_in, *concat_zeros)
        return [
            {nm: np.asarray(out_arrs[i]).reshape(8, *out_avals[i].shape)[c]
             for i, nm in enumerate(out_names)}
            for c in range(8)
        ]

    _CACHED["runner"] = run
    return run


def kernel(x, w_attn, b_attn, w_proj, b_proj):
    x = np.asarray(x, dtype=np.float32)
    w_attn = np.asarray(w_attn, dtype=np.float32)
    b_attn = np.asarray(b_attn, dtype=np.float32)
    w_proj = np.asarray(w_proj, dtype=np.float32)
    b_proj = np.asarray(b_proj, dtype=np.float32)

    in_maps = make_in_maps(x, w_attn, b_attn, w_proj)
    results = None
    try:
        run = _get_runner()
        # The first (cold) execution occasionally races on input
        # streaming and corrupts one core's output (sometimes NaN,
        # sometimes silently). Clean executions are bit-deterministic,
        # so run twice and accept only a matching pair; tie-break with
        # extra runs. Device time per run is ~180us, so this is cheap.
        def _ys(r):
            return np.stack([c["y"].astype(np.float32) for c in r])

        prev = None
        for _ in range(4):
            cur = run(in_maps)
            ycur = _ys(cur)
            if not np.isfinite(ycur).all():
                continue
            if prev is not None and np.array_equal(prev[1], ycur):
                results = cur
                break
            prev = (cur, ycur)
        if results is None and prev is not None:
            results = prev[0]
    except Exception:
        results = None
    if results is None:
        # fallback: the stock SPMD runner (slower per call, same result)
        if "nc" not in _CACHED:
            _CACHED["nc"] = build_nc()
        res = run_bass_kernel_spmd(
            _CACHED["nc"], in_maps, core_ids=list(range(8)))
        results = res.results

    # v-bias contribution: probs rows sum to 1, so attn += 1 * b_v^T, and
    # (1 b_v^T) @ w_proj = row vector b_v @ w_proj added to every position.
    extra = b_attn[2 * C:] @ w_proj + b_proj  # [C]
    out = np.empty((B, T, C), dtype=np.float32)
    for b in range(B):
        out[b] = (results[2 * b]["y"].astype(np.float32)
                  + results[2 * b + 1]["y"].astype(np.float32) + extra)
    return out

